# revision 1
# baseline (speedup 1.0000x reference)
"""Trainium2 Bass kernel for a 2-layer GCN on two graphs (shared weights).

Problem: nn_BRIGHT_gcn (gnn_message_passing).
  reference:
    gcn_conv(x, ei, W, b): deg = 1 + indeg(col); dis = rsqrt(deg)
      h = x @ W; out[c] = sum_{(r,c) in E} dis[r]*dis[c]*h[r] + dis[c]^2*h[c] + b
    two layers, then L1-normalize rows.  Two graphs through the same weights.

Strategy (8 NeuronCores, SPMD):
  - graph g in {0,1} on cores 4g..4g+3; each core owns a contiguous shard of
    25000 destination nodes.
  - Factor the symmetric norm: h' = dis (.) (x @ W).  The edge aggregation is
    then a *plain* segment-sum of h'[src] rows, post-scaled by dis[dst]:
        out = dis (.) (segsum(h'[src] -> dst) + h'[own]) + b
  - NEFF A: h1' = dis (.) (xT.T @ W1) for the core's shard (host passes xT).
  - host: allgather h1' shards -> full table H' per graph.
  - NEFF B (compiled once, run twice): for each 128-dst block, gather the
    incoming edges' h' rows from HBM with dma_gather (int16 idxs, 4 windows of
    32768 rows) and scatter-add them with a one-hot x PE matmul into PSUM.
    Epilogue computes both h_next' = dis (.) ((out+b) @ W2) and l1norm(out+b);
    the host uses h_next' after layer 1 and l1norm after layer 2.

kernel() takes FULL inputs and returns the FULL output tuple.
"""

import math

import numpy as np

P = 128
FEAT = 256
N_NODES = 100000
N_CORES = 8
N_SHARDS = 4  # per graph
SHARD = N_NODES // N_SHARDS  # 25000
NBLK = math.ceil(SHARD / P)  # 196
SHARD_PAD = NBLK * P  # 25088
WIN = 32768  # int16 index window
N_WIN = math.ceil(N_NODES / WIN)  # 4
WIN_SIZES = [min(WIN, N_NODES - w * WIN) for w in range(N_WIN)]  # [32768]*3+[1696]


# ---------------------------------------------------------------------------
# host-side graph preprocessing
# ---------------------------------------------------------------------------

def _prep_graph(edge_index):
    """Degree vector (with self-loops) for one graph."""
    col = np.asarray(edge_index[1], dtype=np.int64)
    deg = np.bincount(col, minlength=N_NODES).astype(np.float32) + 1.0
    dis = (1.0 / np.sqrt(deg)).astype(np.float32)
    return dis


def _prep_shard_edges(edge_index, shard_id):
    """Bucket one shard's incoming edges by (dst block, src window).

    Returns dict with per-(block, window) counts plus sorted per-edge arrays:
      blk   [e] destination block within shard (0..NBLK-1)
      dloc  [e] destination lane within block (0..127)
      widx  [e] source row within its window (0..32767)
      win   [e] source window (0..3)
      cnt   [NBLK, N_WIN] group sizes
    sorted by (blk, win), stable.
    """
    row = np.asarray(edge_index[0], dtype=np.int64)
    col = np.asarray(edge_index[1], dtype=np.int64)
    lo, hi = SHARD * shard_id, SHARD * (shard_id + 1)
    m = (col >= lo) & (col < hi)
    src = row[m]
    dst = col[m] - lo
    blk = dst >> 7
    dloc = dst & 127
    win = src >> 15
    widx = src & (WIN - 1)
    # widx as the innermost key: ascending row addresses within each gather
    # group make the random 1KB reads quasi-sequential in HBM (row-buffer hits)
    order = np.lexsort((widx, win, blk))
    blk, dloc, win, widx = blk[order], dloc[order], win[order], widx[order]
    cnt = np.bincount(blk * N_WIN + win, minlength=NBLK * N_WIN).reshape(NBLK, N_WIN)
    return dict(blk=blk, dloc=dloc, win=win, widx=widx, cnt=cnt)


def _build_core_tables(sh, wc):
    """Build the per-core device-side index/onehot tables.

    wc[w]: chunks (of 128 edges) allotted to every (block, window w) group.
    Layout per block b (free-dim order):
      idx16 : for w in 0..3: wc[w]*128 int16 window-row indices, wrapped
              [16, L/16] (pos j -> partition j%16, slot j//16), replicated to
              128 partitions.
      dstloc: for w in 0..3: wc[w] columns of 128 f32 dst lanes (pad = -1).
    Padding edges gather window row 0 and have dstloc -1 (one-hot zero).
    """
    blk, win, widx, dloc, cnt = sh["blk"], sh["win"], sh["widx"], sh["dloc"], sh["cnt"]
    L = [c * P for c in wc]  # padded group sizes
    SIDX = sum(L) // 16  # int16 cols per block
    NCH = sum(wc)  # chunks per block

    # position of each edge inside its padded (blk, win) slot
    flat_cnt = cnt.reshape(-1)
    starts = np.zeros(NBLK * N_WIN, dtype=np.int64)
    starts[1:] = np.cumsum(flat_cnt)[:-1]
    gid = blk * N_WIN + win
    pos_in_group = np.arange(len(blk)) - starts[gid]
    # base offset of group (b, w) inside the padded per-core layout
    woff = np.zeros(N_WIN, dtype=np.int64)
    for w in range(1, N_WIN):
        woff[w] = woff[w - 1] + L[w - 1]
    pad_base = blk * sum(L) + woff[win]
    flat_pos = pad_base + pos_in_group

    tot = NBLK * sum(L)
    idx_flat = np.zeros(tot, dtype=np.int16)
    dloc_flat = np.full(tot, -1.0, dtype=np.float32)
    idx_flat[flat_pos] = widx.astype(np.int16)
    dloc_flat[flat_pos] = dloc.astype(np.float32)

    # idx16: wrap each (b, w) segment [L] -> [16, L/16], replicate to 128 parts
    idx_bw = idx_flat.reshape(NBLK, sum(L))
    seg_parts = []
    for w in range(N_WIN):
        seg = idx_bw[:, woff[w]:woff[w] + L[w]]  # [NBLK, L[w]]
        seg = seg.reshape(NBLK, L[w] // 16, 16).transpose(2, 0, 1)  # [16, NBLK, L/16]
        seg_parts.append(seg)
    idx16 = np.concatenate(seg_parts, axis=2)  # [16, NBLK, SIDX]
    idx16 = np.tile(idx16, (8, 1, 1)).reshape(P, NBLK * SIDX)

    # dstloc: [128, NBLK * NCH]; column (b, j) = lane values of chunk j
    dl_parts = []
    for w in range(N_WIN):
        seg = dloc_flat.reshape(NBLK, sum(L))[:, woff[w]:woff[w] + L[w]]
        seg = seg.reshape(NBLK, wc[w], P).transpose(2, 0, 1)  # [128, NBLK, wc]
        dl_parts.append(seg)
    import ml_dtypes
    dstloc = np.concatenate(dl_parts, axis=2).reshape(P, NBLK * NCH)
    dstloc = np.ascontiguousarray(dstloc.astype(ml_dtypes.bfloat16))
    return np.ascontiguousarray(idx16), dstloc


def _dis_cols(dis, shard_id):
    """dis for the shard as [128, NBLK] (partition = lane in block)."""
    d = np.zeros(SHARD_PAD, dtype=np.float32)
    d[:SHARD] = dis[SHARD * shard_id:SHARD * (shard_id + 1)]
    return np.ascontiguousarray(d.reshape(NBLK, P).T)


def _host_prep(edge_index1, edge_index2):
    """All static per-core structures. Returns (cores, wc) where cores is a
    list of 8 dicts."""
    dis = [_prep_graph(edge_index1), _prep_graph(edge_index2)]
    shards = []
    for g, ei in enumerate((edge_index1, edge_index2)):
        for s in range(N_SHARDS):
            shards.append((g, s, _prep_shard_edges(ei, s)))
    # global uniform chunk counts per window
    wc = []
    for w in range(N_WIN):
        mx = max(int(sh["cnt"][:, w].max()) for _, _, sh in shards)
        wc.append(max(1, math.ceil(mx / P)))
    cores = []
    for g, s, sh in shards:
        idx16, dstloc = _build_core_tables(sh, wc)
        cores.append(dict(
            graph=g, shard=s,
            idx16=idx16, dstloc=dstloc,
            disb=_dis_cols(dis[g], s),
            dis=dis[g],
        ))
    return cores, wc


# ---------------------------------------------------------------------------
# numpy emulation of the device kernels (for host-side validation)
# ---------------------------------------------------------------------------

def _emu_neff_a(xT, w, disb):
    h = xT.T.astype(np.float32) @ w  # [SHARD_PAD, FEAT]
    return h * disb.T.reshape(SHARD_PAD, 1)


def _emu_neff_b(htab, hown, idx16, dstloc, disb, brep, wnext, wc, nblk=NBLK):
    NBLK = nblk  # noqa: N806
    SHARD_PAD = NBLK * P  # noqa: N806
    SIDX = sum(wc) * 8
    NCH = sum(wc)
    if htab.shape[1] == 2 * FEAT:  # bf16x2 hi|lo packed rows
        htab = htab[:, :FEAT].astype(np.float32) + htab[:, FEAT:].astype(np.float32)
    hnext = np.zeros((SHARD_PAD, FEAT), np.float32)
    normout = np.zeros((SHARD_PAD, FEAT), np.float32)
    w0 = [0]
    for w in range(1, N_WIN):
        w0.append(w0[-1] + wc[w - 1] * 8)
    for b in range(NBLK):
        idxs = idx16[:16, b * SIDX:(b + 1) * SIDX]
        acc = np.zeros((P, FEAT), np.float32)
        j = 0
        for w in range(N_WIN):
            seg = idxs[:, w0[w]:w0[w] + wc[w] * 8]  # [16, wc*8]
            unwrapped = seg.T.reshape(-1)  # pos j at (j%16, j//16) -> j = s*16+p
            gathered = htab[w * WIN + unwrapped.astype(np.int64)]  # [wc*128, FEAT]
            for jw in range(wc[w]):
                chunk = gathered[jw * P:(jw + 1) * P]  # [128, FEAT]
                dl = dstloc[:, b * NCH + j]  # [128]
                onehot = (dl[:, None] == np.arange(P)[None, :]).astype(np.float32)
                acc += onehot.T @ chunk
                j += 1
        t1 = acc + hown[b * P:(b + 1) * P]
        out_layer = t1 * disb[:, b:b + 1] + brep
        hn = (out_layer @ wnext) * disb[:, b:b + 1]
        hnext[b * P:(b + 1) * P] = hn
        s = np.maximum(np.abs(out_layer).sum(1, keepdims=True), 1e-12)
        normout[b * P:(b + 1) * P] = out_layer / s
    return hnext, normout


# ---------------------------------------------------------------------------
# device kernels (bass/tile)
# ---------------------------------------------------------------------------

F32 = None  # filled lazily (mybir import)


def _build_neff_a():
    import concourse.bacc as bacc
    import concourse.mybir as mybir
    import concourse.tile as tile

    f32 = mybir.dt.float32
    nc = bacc.Bacc("TRN2", target_bir_lowering=False, debug=False)
    xT = nc.dram_tensor("xT", [FEAT, SHARD_PAD], f32, kind="ExternalInput")
    w_in = nc.dram_tensor("w", [FEAT, FEAT], f32, kind="ExternalInput")
    disb = nc.dram_tensor("disb", [P, NBLK], f32, kind="ExternalInput")
    hout = nc.dram_tensor("hout", [SHARD_PAD, FEAT], f32, kind="ExternalOutput")

    with tile.TileContext(nc) as tc:
        with (
            tc.tile_pool(name="const", bufs=1) as cpool,
            tc.tile_pool(name="work", bufs=3) as wpool,
            tc.tile_pool(name="psum", bufs=2, space="PSUM") as ppool,
        ):
            w_sb = cpool.tile([P, 2, FEAT], f32, tag="w")
            nc.sync.dma_start(out=w_sb[:, 0, :], in_=w_in[0:P, :])
            nc.sync.dma_start(out=w_sb[:, 1, :], in_=w_in[P:FEAT, :])
            dis_sb = cpool.tile([P, NBLK], f32, tag="dis")
            nc.sync.dma_start(out=dis_sb[:], in_=disb[:, :])
            for t in range(NBLK):
                xt = wpool.tile([P, 2, P], f32, tag="xT")
                nc.sync.dma_start(out=xt[:, 0, :], in_=xT[0:P, t * P:(t + 1) * P])
                nc.sync.dma_start(out=xt[:, 1, :], in_=xT[P:FEAT, t * P:(t + 1) * P])
                ps = ppool.tile([P, FEAT], f32, tag="ps")
                nc.tensor.matmul(ps[:], lhsT=xt[:, 0, :], rhs=w_sb[:, 0, :],
                                 start=True, stop=False)
                nc.tensor.matmul(ps[:], lhsT=xt[:, 1, :], rhs=w_sb[:, 1, :],
                                 start=False, stop=True)
                hs = wpool.tile([P, FEAT], f32, tag="hs")
                nc.vector.tensor_scalar(out=hs[:], in0=ps[:],
                                        scalar1=dis_sb[:, t:t + 1], scalar2=None,
                                        op0=mybir.AluOpType.mult)
                nc.sync.dma_start(out=hout[t * P:(t + 1) * P, :], in_=hs[:])
    nc.compile()
    return nc


def _build_neff_b(wc, agg_dtype="float32", nblk=NBLK):
    import concourse.bacc as bacc
    import concourse.mybir as mybir
    import concourse.tile as tile
    from concourse.masks import make_identity

    NBLK = nblk  # noqa: N806 — shadow module constant for size-reduced builds
    SHARD_PAD = NBLK * P  # noqa: N806
    f32 = mybir.dt.float32
    hilo = agg_dtype == "bf16x2"
    agg_dt = mybir.dt.bfloat16 if hilo else getattr(mybir.dt, agg_dtype)
    row_elems = 2 * FEAT if hilo else FEAT  # hi|lo packed rows vs plain f32
    SIDX = sum(wc) * 8
    NCH = sum(wc)
    nc = bacc.Bacc("TRN2", target_bir_lowering=False, debug=False)
    htab = nc.dram_tensor("htab", [N_NODES, row_elems], agg_dt if hilo else f32,
                          kind="ExternalInput")
    hown = nc.dram_tensor("hown", [SHARD_PAD, FEAT], f32, kind="ExternalInput")
    idx16 = nc.dram_tensor("idx16", [P, NBLK * SIDX], mybir.dt.int16,
                           kind="ExternalInput")
    dstloc = nc.dram_tensor("dstloc", [P, NBLK * NCH], mybir.dt.bfloat16,
                            kind="ExternalInput")
    disb = nc.dram_tensor("disb", [P, NBLK], f32, kind="ExternalInput")
    brep = nc.dram_tensor("brep", [P, FEAT], f32, kind="ExternalInput")
    bf16 = mybir.dt.bfloat16
    OHG = 8  # one-hot chunks generated per DVE op
    iotaf = nc.dram_tensor("iotaf", [P, OHG * P], bf16, kind="ExternalInput")
    wnext = nc.dram_tensor("wnext", [FEAT, FEAT], f32, kind="ExternalInput")
    hnext = nc.dram_tensor("hnext", [SHARD_PAD, FEAT], f32, kind="ExternalOutput")
    normout = nc.dram_tensor("normout", [SHARD_PAD, FEAT], f32,
                             kind="ExternalOutput")

    # free-dim offsets of window segments inside a block's idx16 slice
    ioff = [0]
    for w in range(1, N_WIN):
        ioff.append(ioff[-1] + wc[w - 1] * 8)

    with tile.TileContext(nc) as tc:
        with (
            tc.tile_pool(name="const", bufs=1) as cpool,
            tc.tile_pool(name="gland", bufs=2) as gpool,
            tc.tile_pool(name="work", bufs=3) as wpool,
            tc.tile_pool(name="oh", bufs=10) as ohpool,
            tc.tile_pool(name="psum", bufs=2, space="PSUM") as ppool,
            tc.tile_pool(name="psumt", bufs=2, space="PSUM") as ptpool,
        ):
            w_sb = cpool.tile([P, 2, FEAT], f32, tag="w")
            nc.sync.dma_start(out=w_sb[:, 0, :], in_=wnext[0:P, :])
            nc.sync.dma_start(out=w_sb[:, 1, :], in_=wnext[P:FEAT, :])
            dis_sb = cpool.tile([P, NBLK], f32, tag="dis")
            nc.sync.dma_start(out=dis_sb[:], in_=disb[:, :])
            b_sb = cpool.tile([P, FEAT], f32, tag="b")
            nc.sync.dma_start(out=b_sb[:], in_=brep[:, :])
            iota_sb = cpool.tile([P, OHG, P], bf16, tag="iota")
            nc.sync.dma_start(out=iota_sb[:], in_=iotaf[:, :].rearrange(
                "p (g q) -> p g q", g=OHG))
            dl_sb = cpool.tile([P, NBLK * NCH], bf16, tag="dl")
            nc.sync.dma_start(out=dl_sb[:], in_=dstloc[:, :])
            ident = cpool.tile([P, P], f32, tag="ident")
            make_identity(nc, ident[:])

            for b in range(NBLK):
                idxt = wpool.tile([P, SIDX], mybir.dt.int16, tag="idx")
                nc.sync.dma_start(out=idxt[:],
                                  in_=idx16[:, b * SIDX:(b + 1) * SIDX])
                glands = []
                for w in range(N_WIN):
                    gt = gpool.tile([P, wc[w], row_elems], agg_dt, tag=f"g{w}")
                    src = htab[w * WIN:w * WIN + WIN_SIZES[w], :]
                    if not hilo:
                        src = src.bitcast(agg_dt)
                    # the SWDGE descriptor ring caps one call at 1024 rows
                    for p0 in range(0, wc[w], 8):
                        pc = min(8, wc[w] - p0)
                        nc.gpsimd.dma_gather(
                            gt[:, p0:p0 + pc, :],
                            src,
                            idxt[:, ioff[w] + p0 * 8:ioff[w] + (p0 + pc) * 8],
                            pc * P,
                            pc * P,
                            row_elems,
                            single_packet=False,
                        )
                    glands.append(gt)
                ps = ppool.tile([P, FEAT], f32, tag="agg")
                # batched one-hot generation: OHG chunks per DVE op
                ohs = []
                for c0 in range(0, NCH, OHG):
                    g = min(OHG, NCH - c0)
                    oh = ohpool.tile([P, OHG, P], agg_dt, tag="oh")
                    nc.vector.tensor_tensor(
                        out=oh[:, :g, :], in0=iota_sb[:, :g, :],
                        in1=dl_sb[:, b * NCH + c0:b * NCH + c0 + g]
                        .to_broadcast([P, g, P]),
                        op=mybir.AluOpType.is_equal)
                    ohs.append(oh)
                j = 0
                for w in range(N_WIN):
                    for jw in range(wc[w]):
                        oh = ohs[j // OHG][:, j % OHG, :]
                        if hilo:
                            nc.tensor.matmul(ps[:], lhsT=oh,
                                             rhs=glands[w][:, jw, 0:FEAT],
                                             start=(j == 0), stop=False)
                            nc.tensor.matmul(ps[:], lhsT=oh,
                                             rhs=glands[w][:, jw, FEAT:2 * FEAT],
                                             start=False, stop=(j == NCH - 1))
                        else:
                            nc.tensor.matmul(ps[:], lhsT=oh,
                                             rhs=glands[w][:, jw, :],
                                             start=(j == 0), stop=(j == NCH - 1))
                        j += 1
                hot = wpool.tile([P, FEAT], f32, tag="hot")
                nc.sync.dma_start(out=hot[:], in_=hown[b * P:(b + 1) * P, :])
                t1 = wpool.tile([P, FEAT], f32, tag="t1")
                nc.vector.tensor_tensor(out=t1[:], in0=ps[:], in1=hot[:],
                                        op=mybir.AluOpType.add)
                t2 = wpool.tile([P, FEAT], f32, tag="t2")
                nc.vector.tensor_scalar(out=t2[:], in0=t1[:],
                                        scalar1=dis_sb[:, b:b + 1], scalar2=None,
                                        op0=mybir.AluOpType.mult)
                outl = wpool.tile([P, FEAT], f32, tag="outl")
                nc.vector.tensor_tensor(out=outl[:], in0=t2[:], in1=b_sb[:],
                                        op=mybir.AluOpType.add)
                # --- h_next branch: dis (.) (outl @ W2)
                tp = ptpool.tile([P, 2, P], f32, tag="tp")
                nc.tensor.transpose(tp[:, 0, :], outl[:, 0:P], ident[:])
                nc.tensor.transpose(tp[:, 1, :], outl[:, P:FEAT], ident[:])
                tts = wpool.tile([P, 2, P], f32, tag="tts")
                nc.vector.tensor_copy(tts[:, 0, :], tp[:, 0, :])
                nc.vector.tensor_copy(tts[:, 1, :], tp[:, 1, :])
                ps2 = ppool.tile([P, FEAT], f32, tag="mm2")
                nc.tensor.matmul(ps2[:], lhsT=tts[:, 0, :], rhs=w_sb[:, 0, :],
                                 start=True, stop=False)
                nc.tensor.matmul(ps2[:], lhsT=tts[:, 1, :], rhs=w_sb[:, 1, :],
                                 start=False, stop=True)
                hn = wpool.tile([P, FEAT], f32, tag="hn")
                nc.vector.tensor_scalar(out=hn[:], in0=ps2[:],
                                        scalar1=dis_sb[:, b:b + 1], scalar2=None,
                                        op0=mybir.AluOpType.mult)
                nc.sync.dma_start(out=hnext[b * P:(b + 1) * P, :], in_=hn[:])
                # --- l1 normalize branch
                s1 = wpool.tile([P, 1], f32, tag="s1")
                nc.vector.tensor_reduce(out=s1[:], in_=outl[:],
                                        axis=mybir.AxisListType.X,
                                        op=mybir.AluOpType.add,
                                        apply_absolute_value=True)
                s2 = wpool.tile([P, 1], f32, tag="s2")
                nc.vector.tensor_scalar(out=s2[:], in0=s1[:], scalar1=1e-12,
                                        scalar2=None, op0=mybir.AluOpType.max)
                rs = wpool.tile([P, 1], f32, tag="rs")
                nc.vector.reciprocal(rs[:], s2[:])
                no = wpool.tile([P, FEAT], f32, tag="no")
                nc.vector.tensor_scalar(out=no[:], in0=outl[:],
                                        scalar1=rs[:, 0:1], scalar2=None,
                                        op0=mybir.AluOpType.mult)
                nc.sync.dma_start(out=normout[b * P:(b + 1) * P, :], in_=no[:])
    nc.compile()
    return nc


# ---------------------------------------------------------------------------
# orchestration
# ---------------------------------------------------------------------------

RUN_INFO = []  # per-launch {name, wall_s, exec_time_ns} (exec only when traced)

AGG_MODE = "float32"  # "float32" (exact, 4 cyc/row) | "bf16x2" (hi/lo split,
#                       ~1.5e-5 rel err, 2x1 cyc/row) | "float32r" (~2e-4)
# NOTE: the launch-B bottleneck is GPSIMD descriptor generation for the
# per-edge dma_gathers (~99% busy), so the cheaper PE modes don't pay for
# their precision loss; float32 is exact and the same wall time.


def _pack_hilo(h):
    """f32 [N, FEAT] -> bf16 [N, 2*FEAT] with hi|lo split (hi+lo ~= h)."""
    import ml_dtypes
    hi = h.astype(ml_dtypes.bfloat16)
    lo = (h - hi.astype(np.float32)).astype(ml_dtypes.bfloat16)
    out = np.empty((h.shape[0], 2 * FEAT), dtype=ml_dtypes.bfloat16)
    out[:, :FEAT] = hi
    out[:, FEAT:] = lo
    return out


_IOTA = None


def _iota_tile():
    global _IOTA
    if _IOTA is None:
        import ml_dtypes
        _IOTA = np.ascontiguousarray(np.broadcast_to(
            np.arange(P, dtype=np.float32).astype(ml_dtypes.bfloat16),
            (P, 8, P)).reshape(P, 8 * P))
    return _IOTA


def _pad_rows(a, n):
    out = np.zeros((n, a.shape[1]), dtype=a.dtype)
    out[:a.shape[0]] = a
    return out


def kernel(x1, x2, edge_index1, edge_index2, W1, b1, W2, b2, _emulate=False):
    from concourse.bass_utils import run_bass_kernel_spmd

    x = [np.asarray(x1, np.float32), np.asarray(x2, np.float32)]
    W1 = np.asarray(W1, np.float32)
    W2 = np.asarray(W2, np.float32)
    b1 = np.asarray(b1, np.float32)
    b2 = np.asarray(b2, np.float32)
    cores, wc = _host_prep(np.asarray(edge_index1), np.asarray(edge_index2))
    core_ids = list(range(N_CORES))

    # ---- launch A: h1' = dis (.) (x @ W1) per shard
    a_maps = []
    for c in cores:
        g, s = c["graph"], c["shard"]
        xs = _pad_rows(x[g][SHARD * s:SHARD * (s + 1)], SHARD_PAD)
        a_maps.append(dict(xT=np.ascontiguousarray(xs.T), w=W1, disb=c["disb"]))

    import time

    def _run(nc, maps, name):
        t0 = time.time()
        res = run_bass_kernel_spmd(nc, maps, core_ids)
        RUN_INFO.append(dict(name=name, wall_s=time.time() - t0,
                             exec_time_ns=res.exec_time_ns,
                             profile=res.profile_json))
        return res.results

    if _emulate:
        a_out = [dict(hout=_emu_neff_a(m["xT"], m["w"], m["disb"]))
                 for m in a_maps]
    else:
        nc_a = _build_neff_a()
        a_out = _run(nc_a, a_maps, "A")

    def full_table(outs, key):
        tabs = []
        for g in range(2):
            shards = [outs[g * N_SHARDS + s][key][:SHARD] for s in range(N_SHARDS)]
            tabs.append(np.ascontiguousarray(np.concatenate(shards, axis=0)))
        return tabs

    h1 = full_table(a_out, "hout")

    # ---- launch B (x2): aggregation layers
    nc_b = None if _emulate else _build_neff_b(wc, agg_dtype=AGG_MODE)

    def run_b(htabs, bias):
        maps = []
        brep = np.ascontiguousarray(np.broadcast_to(bias, (P, FEAT)))
        if AGG_MODE == "bf16x2":
            packed = [_pack_hilo(t) for t in htabs]
            # self-loop term must match what the gathers see: hi + lo
            recon = [p[:, :FEAT].astype(np.float32) + p[:, FEAT:].astype(np.float32)
                     for p in packed]
        else:
            packed = recon = htabs
        for c in cores:
            g, s = c["graph"], c["shard"]
            maps.append(dict(
                htab=packed[g],
                hown=_pad_rows(recon[g][SHARD * s:SHARD * (s + 1)], SHARD_PAD),
                idx16=c["idx16"], dstloc=c["dstloc"], disb=c["disb"],
                brep=brep, iotaf=_iota_tile(), wnext=W2,
            ))
        if _emulate:
            return [dict(zip(("hnext", "normout"),
                             _emu_neff_b(m["htab"], m["hown"], m["idx16"],
                                         m["dstloc"], m["disb"], m["brep"],
                                         m["wnext"], wc)))
                    for m in maps]
        run_b.n = getattr(run_b, "n", 0) + 1
        return _run(nc_b, maps, f"B{run_b.n}")

    b1_out = run_b(h1, b1)
    h2 = full_table(b1_out, "hnext")
    b2_out = run_b(h2, b2)
    emd = full_table(b2_out, "normout")
    return emd[0], emd[1]



# revision 3
# speedup vs baseline: 2.4150x; 2.4150x over previous
"""Trainium2 Bass kernel for a 2-layer GCN on two graphs (shared weights).

Problem: nn_BRIGHT_gcn (gnn_message_passing).
  reference:
    gcn_conv(x, ei, W, b): deg = 1 + indeg(col); dis = rsqrt(deg)
      h = x @ W; out[c] = sum_{(r,c) in E} dis[r]*dis[c]*h[r] + dis[c]^2*h[c] + b
    two layers, then L1-normalize rows.  Two graphs through the same weights.

Strategy (8 NeuronCores, SPMD):
  - graph g in {0,1} on cores 4g..4g+3; each core owns a contiguous shard of
    25000 destination nodes.
  - Factor the symmetric norm: h' = dis (.) (x @ W) stored as a bf16 table.
    The edge aggregation is a plain segment-sum of h'[src] rows, post-scaled
    by dis[dst]:  out = dis (.) (segsum(h'[src] -> dst) + h'[own]) + b
  - NEFF A: h1' = dis (.) (xT.T @ W1) in bf16 for the core's shard.
  - host: allgather h1' shards -> full bf16 table H' per graph (free).
  - NEFF B (compiled once, run twice): for each 128-dst block, gather the
    incoming edges' bf16 h' rows from HBM with dma_gather (int16 idxs, 4
    windows of 32768 rows, 4 SWDGE queues round-robin so Q7 descriptor
    generation overlaps ~2x) and scatter-add them with one-hot x PE matmuls
    (bf16, 1 cyc/row) into f32 PSUM.  Epilogue computes both
    h_next' = dis (.) ((out+b) @ W2) (bf16, the next layer's table) and
    l1norm(out+b) (f32); the host uses h_next' after layer 1 and l1norm
    after layer 2.
  - chunk counts are per-(block, window) maxima over the 8 cores (not a
    global max), cutting gather padding from +16% to +8%.

kernel() takes FULL inputs and returns the FULL output tuple.
"""

import math

import numpy as np

P = 128
FEAT = 256
N_NODES = 100000
N_CORES = 8
N_SHARDS = 4  # per graph
SHARD = N_NODES // N_SHARDS  # 25000
NBLK = math.ceil(SHARD / P)  # 196
SHARD_PAD = NBLK * P  # 25088
WIN = 32768  # int16 index window
N_WIN = math.ceil(N_NODES / WIN)  # 4
WIN_SIZES = [min(WIN, N_NODES - w * WIN) for w in range(N_WIN)]  # [32768]*3+[1696]
N_QUEUES = 4  # SWDGE descriptor queues (round-robin over gather calls)


# ---------------------------------------------------------------------------
# host-side graph preprocessing
# ---------------------------------------------------------------------------

def _prep_graph(edge_index):
    """Degree vector (with self-loops) for one graph."""
    col = np.asarray(edge_index[1], dtype=np.int64)
    deg = np.bincount(col, minlength=N_NODES).astype(np.float32) + 1.0
    dis = (1.0 / np.sqrt(deg)).astype(np.float32)
    return dis


def _prep_shard_edges(edge_index, shard_id):
    """Bucket one shard's incoming edges by (dst block, src window).

    Returns dict with per-(block, window) counts plus sorted per-edge arrays:
      blk   [e] destination block within shard (0..NBLK-1)
      dloc  [e] destination lane within block (0..127)
      widx  [e] source row within its window (0..32767)
      win   [e] source window (0..3)
      cnt   [NBLK, N_WIN] group sizes
    sorted by (blk, win), stable.
    """
    row = np.asarray(edge_index[0], dtype=np.int64)
    col = np.asarray(edge_index[1], dtype=np.int64)
    lo, hi = SHARD * shard_id, SHARD * (shard_id + 1)
    m = (col >= lo) & (col < hi)
    src = row[m]
    dst = col[m] - lo
    blk = dst >> 7
    dloc = dst & 127
    win = src >> 15
    widx = src & (WIN - 1)
    # widx as the innermost key: ascending row addresses within each gather
    # group make the random reads quasi-sequential in HBM (row-buffer hits)
    order = np.lexsort((widx, win, blk))
    blk, dloc, win, widx = blk[order], dloc[order], win[order], widx[order]
    cnt = np.bincount(blk * N_WIN + win, minlength=NBLK * N_WIN).reshape(NBLK, N_WIN)
    return dict(blk=blk, dloc=dloc, win=win, widx=widx, cnt=cnt)


def _build_core_tables(sh, wcb):
    """Build the per-core device-side index/onehot tables.

    wcb[b, w]: chunks (of 128 edges) allotted to the (block b, window w)
    group — per-(b, w) maxima over cores, shared by the single NEFF.
    Flat free-dim layout per block b:
      idx16 : for w in 0..3: wcb[b,w]*128 int16 window-row indices, wrapped
              [16, L/16] (pos j -> partition j%16, slot j//16), replicated to
              128 partitions.
      dstloc: for w in 0..3: wcb[b,w] columns of 128 bf16 dst lanes (pad=-1).
    Padding edges gather window row 0 and have dstloc -1 (one-hot zero).
    """
    import ml_dtypes

    blk, win, widx, dloc, cnt = sh["blk"], sh["win"], sh["widx"], sh["dloc"], sh["cnt"]
    L = wcb * P  # [NBLK, N_WIN] padded group sizes
    # free-dim offset of group (b, w) in the flat per-edge layout
    flat_sizes = L.reshape(-1)
    gstart = np.zeros(NBLK * N_WIN, dtype=np.int64)
    gstart[1:] = np.cumsum(flat_sizes)[:-1]
    tot = int(flat_sizes.sum())

    flat_cnt = cnt.reshape(-1)
    estart = np.zeros(NBLK * N_WIN, dtype=np.int64)
    estart[1:] = np.cumsum(flat_cnt)[:-1]
    gid = blk * N_WIN + win
    pos_in_group = np.arange(len(blk)) - estart[gid]
    flat_pos = gstart[gid] + pos_in_group

    idx_flat = np.zeros(tot, dtype=np.int16)
    dloc_flat = np.full(tot, -1.0, dtype=np.float32)
    idx_flat[flat_pos] = widx.astype(np.int16)
    dloc_flat[flat_pos] = dloc.astype(np.float32)

    # idx16: per (b, w) wrap [L] -> [16, L/16]; concat along free dim;
    # replicate to 128 partitions.
    idx_parts = []
    dl_parts = []
    for b in range(NBLK):
        for w in range(N_WIN):
            g0 = gstart[b * N_WIN + w]
            seg = idx_flat[g0:g0 + L[b, w]]
            idx_parts.append(seg.reshape(L[b, w] // 16, 16).T)  # [16, L/16]
            dl_parts.append(dloc_flat[g0:g0 + L[b, w]].reshape(wcb[b, w], P).T)
    idx16 = np.concatenate(idx_parts, axis=1)  # [16, tot/16]
    idx16 = np.tile(idx16, (8, 1))  # [128, tot/16]
    dstloc = np.concatenate(dl_parts, axis=1).astype(ml_dtypes.bfloat16)
    return (np.ascontiguousarray(idx16), np.ascontiguousarray(dstloc))


def _dis_cols(dis, shard_id):
    """dis for the shard as [128, NBLK] (partition = lane in block)."""
    d = np.zeros(SHARD_PAD, dtype=np.float32)
    d[:SHARD] = dis[SHARD * shard_id:SHARD * (shard_id + 1)]
    return np.ascontiguousarray(d.reshape(NBLK, P).T)


def _host_prep(edge_index1, edge_index2):
    """All static per-core structures. Returns (cores, wcb)."""
    dis = [_prep_graph(edge_index1), _prep_graph(edge_index2)]
    shards = []
    for g, ei in enumerate((edge_index1, edge_index2)):
        for s in range(N_SHARDS):
            shards.append((g, s, _prep_shard_edges(ei, s)))
    # per-(block, window) chunk counts: max over the 8 cores
    cnt_max = np.stack([sh["cnt"] for _, _, sh in shards]).max(axis=0)
    wcb = np.maximum(1, np.ceil(cnt_max / P).astype(np.int64))  # [NBLK, N_WIN]
    cores = []
    for g, s, sh in shards:
        idx16, dstloc = _build_core_tables(sh, wcb)
        cores.append(dict(
            graph=g, shard=s,
            idx16=idx16, dstloc=dstloc,
            disb=_dis_cols(dis[g], s),
            dis=dis[g],
        ))
    return cores, wcb


# ---------------------------------------------------------------------------
# device kernels (bass/tile)
# ---------------------------------------------------------------------------

def _build_neff_a():
    import concourse.bacc as bacc
    import concourse.mybir as mybir
    import concourse.tile as tile

    f32 = mybir.dt.float32
    bf16 = mybir.dt.bfloat16
    nc = bacc.Bacc("TRN2", target_bir_lowering=False, debug=False)
    xT = nc.dram_tensor("xT", [FEAT, SHARD_PAD], bf16, kind="ExternalInput")
    w_in = nc.dram_tensor("w", [FEAT, FEAT], bf16, kind="ExternalInput")
    disb = nc.dram_tensor("disb", [P, NBLK], f32, kind="ExternalInput")
    hout = nc.dram_tensor("hout", [SHARD_PAD, FEAT], bf16, kind="ExternalOutput")

    with tile.TileContext(nc) as tc:
        with (
            tc.tile_pool(name="const", bufs=1) as cpool,
            tc.tile_pool(name="work", bufs=3) as wpool,
            tc.tile_pool(name="psum", bufs=2, space="PSUM") as ppool,
        ):
            w_sb = cpool.tile([P, 2, FEAT], bf16, tag="w")
            nc.sync.dma_start(out=w_sb[:, 0, :], in_=w_in[0:P, :])
            nc.sync.dma_start(out=w_sb[:, 1, :], in_=w_in[P:FEAT, :])
            dis_sb = cpool.tile([P, NBLK], f32, tag="dis")
            nc.sync.dma_start(out=dis_sb[:], in_=disb[:, :])
            for t in range(NBLK):
                xt = wpool.tile([P, 2, P], bf16, tag="xT")
                nc.sync.dma_start(out=xt[:, 0, :], in_=xT[0:P, t * P:(t + 1) * P])
                nc.sync.dma_start(out=xt[:, 1, :], in_=xT[P:FEAT, t * P:(t + 1) * P])
                ps = ppool.tile([P, FEAT], f32, tag="ps")
                nc.tensor.matmul(ps[:], lhsT=xt[:, 0, :], rhs=w_sb[:, 0, :],
                                 start=True, stop=False)
                nc.tensor.matmul(ps[:], lhsT=xt[:, 1, :], rhs=w_sb[:, 1, :],
                                 start=False, stop=True)
                hs = wpool.tile([P, FEAT], bf16, tag="hs")
                nc.vector.tensor_scalar(out=hs[:], in0=ps[:],
                                        scalar1=dis_sb[:, t:t + 1], scalar2=None,
                                        op0=mybir.AluOpType.mult)
                nc.sync.dma_start(out=hout[t * P:(t + 1) * P, :], in_=hs[:])
    nc.compile()
    return nc


def _build_neff_b(wcb):
    import concourse.bacc as bacc
    import concourse.mybir as mybir
    import concourse.tile as tile
    from concourse.masks import make_identity

    f32 = mybir.dt.float32
    bf16 = mybir.dt.bfloat16
    i16 = mybir.dt.int16
    SIDXb = [int(wcb[b].sum()) * 8 for b in range(NBLK)]  # int16 cols per block
    NCHb = [int(wcb[b].sum()) for b in range(NBLK)]  # chunks per block
    TOT_SIDX = sum(SIDXb)
    TOT_NCH = sum(NCHb)
    SIDX_MAX = max(SIDXb)
    NCH_MAX = max(NCHb)
    nc = bacc.Bacc("TRN2", target_bir_lowering=False, debug=False,
                   num_swdge_queues=N_QUEUES)
    htab = nc.dram_tensor("htab", [N_NODES, FEAT], bf16, kind="ExternalInput")
    hown = nc.dram_tensor("hown", [SHARD_PAD, FEAT], bf16, kind="ExternalInput")
    idx16 = nc.dram_tensor("idx16", [P, TOT_SIDX], i16, kind="ExternalInput")
    dstloc = nc.dram_tensor("dstloc", [P, TOT_NCH], bf16, kind="ExternalInput")
    disb = nc.dram_tensor("disb", [P, NBLK], f32, kind="ExternalInput")
    brep = nc.dram_tensor("brep", [P, FEAT], f32, kind="ExternalInput")
    OHG = 8  # one-hot chunks generated per DVE op
    iotaf = nc.dram_tensor("iotaf", [P, OHG * P], bf16, kind="ExternalInput")
    wnext = nc.dram_tensor("wnext", [FEAT, FEAT], bf16, kind="ExternalInput")
    hnext = nc.dram_tensor("hnext", [SHARD_PAD, FEAT], bf16,
                           kind="ExternalOutput")
    normout = nc.dram_tensor("normout", [SHARD_PAD, FEAT], f32,
                             kind="ExternalOutput")

    qc = [0]  # SWDGE queue rotation counter

    def next_q():
        q = qc[0] % N_QUEUES
        qc[0] += 1
        return q

    with tile.TileContext(nc) as tc:
        with (
            tc.tile_pool(name="const", bufs=1) as cpool,
            tc.tile_pool(name="gland", bufs=2) as gpool,
            tc.tile_pool(name="work", bufs=3) as wpool,
            tc.tile_pool(name="oh", bufs=10) as ohpool,
            tc.tile_pool(name="psum", bufs=2, space="PSUM") as ppool,
            tc.tile_pool(name="psumt", bufs=2, space="PSUM") as ptpool,
        ):
            w_sb = cpool.tile([P, 2, FEAT], bf16, tag="w")
            nc.sync.dma_start(out=w_sb[:, 0, :], in_=wnext[0:P, :])
            nc.sync.dma_start(out=w_sb[:, 1, :], in_=wnext[P:FEAT, :])
            dis_sb = cpool.tile([P, NBLK], f32, tag="dis")
            nc.sync.dma_start(out=dis_sb[:], in_=disb[:, :])
            b_sb = cpool.tile([P, FEAT], f32, tag="b")
            nc.sync.dma_start(out=b_sb[:], in_=brep[:, :])
            iota_sb = cpool.tile([P, OHG, P], bf16, tag="iota")
            nc.sync.dma_start(out=iota_sb[:], in_=iotaf[:, :].rearrange(
                "p (g q) -> p g q", g=OHG))
            dl_sb = cpool.tile([P, TOT_NCH], bf16, tag="dl")
            nc.sync.dma_start(out=dl_sb[:], in_=dstloc[:, :])
            ident = cpool.tile([P, P], bf16, tag="ident")
            make_identity(nc, ident[:])

            ioff_b = 0  # running idx16 free-dim offset
            coff_b = 0  # running dstloc chunk-col offset
            for b in range(NBLK):
                SIDX = SIDXb[b]
                NCH = NCHb[b]
                idxt = wpool.tile([P, SIDX_MAX], i16, tag="idx")
                nc.sync.dma_start(out=idxt[:, 0:SIDX],
                                  in_=idx16[:, ioff_b:ioff_b + SIDX])
                gt = gpool.tile([P, NCH_MAX, FEAT], bf16, tag="g")
                ioff_w = 0
                goff = 0
                for w in range(N_WIN):
                    wc = int(wcb[b, w])
                    src = htab[w * WIN:w * WIN + WIN_SIZES[w], :]
                    for p0 in range(0, wc, 8):
                        pc = min(8, wc - p0)
                        nc.gpsimd.dma_gather(
                            gt[:, goff + p0:goff + p0 + pc, :],
                            src,
                            idxt[:, ioff_w + p0 * 8:ioff_w + (p0 + pc) * 8],
                            pc * P,
                            pc * P,
                            FEAT,
                            single_packet=False,
                            queue_num=next_q(),
                        )
                    ioff_w += wc * 8
                    goff += wc
                ps = ppool.tile([P, FEAT], f32, tag="agg")
                # batched one-hot generation: OHG chunks per DVE op
                ohs = []
                for c0 in range(0, NCH, OHG):
                    g = min(OHG, NCH - c0)
                    oh = ohpool.tile([P, OHG, P], bf16, tag="oh")
                    nc.vector.tensor_tensor(
                        out=oh[:, :g, :], in0=iota_sb[:, :g, :],
                        in1=dl_sb[:, coff_b + c0:coff_b + c0 + g]
                        .to_broadcast([P, g, P]),
                        op=mybir.AluOpType.is_equal)
                    ohs.append(oh)
                for j in range(NCH):
                    oh = ohs[j // OHG][:, j % OHG, :]
                    nc.tensor.matmul(ps[:], lhsT=oh, rhs=gt[:, j, :],
                                     start=(j == 0), stop=(j == NCH - 1))
                hot = wpool.tile([P, FEAT], bf16, tag="hot")
                nc.sync.dma_start(out=hot[:], in_=hown[b * P:(b + 1) * P, :])
                t1 = wpool.tile([P, FEAT], f32, tag="t1")
                nc.vector.tensor_tensor(out=t1[:], in0=ps[:], in1=hot[:],
                                        op=mybir.AluOpType.add)
                t2 = wpool.tile([P, FEAT], f32, tag="t2")
                nc.vector.tensor_scalar(out=t2[:], in0=t1[:],
                                        scalar1=dis_sb[:, b:b + 1], scalar2=None,
                                        op0=mybir.AluOpType.mult)
                outl = wpool.tile([P, FEAT], f32, tag="outl")
                nc.vector.tensor_tensor(out=outl[:], in0=t2[:], in1=b_sb[:],
                                        op=mybir.AluOpType.add)
                # --- h_next branch: dis (.) (outl @ W2), bf16
                ob = wpool.tile([P, FEAT], bf16, tag="ob")
                nc.vector.tensor_copy(ob[:], outl[:])
                tp = ptpool.tile([P, 2, P], bf16, tag="tp")
                nc.tensor.transpose(tp[:, 0, :], ob[:, 0:P], ident[:])
                nc.tensor.transpose(tp[:, 1, :], ob[:, P:FEAT], ident[:])
                tts = wpool.tile([P, 2, P], bf16, tag="tts")
                nc.vector.tensor_copy(tts[:, 0, :], tp[:, 0, :])
                nc.vector.tensor_copy(tts[:, 1, :], tp[:, 1, :])
                ps2 = ppool.tile([P, FEAT], f32, tag="mm2")
                nc.tensor.matmul(ps2[:], lhsT=tts[:, 0, :], rhs=w_sb[:, 0, :],
                                 start=True, stop=False)
                nc.tensor.matmul(ps2[:], lhsT=tts[:, 1, :], rhs=w_sb[:, 1, :],
                                 start=False, stop=True)
                hn = wpool.tile([P, FEAT], bf16, tag="hn")
                nc.vector.tensor_scalar(out=hn[:], in0=ps2[:],
                                        scalar1=dis_sb[:, b:b + 1], scalar2=None,
                                        op0=mybir.AluOpType.mult)
                nc.sync.dma_start(out=hnext[b * P:(b + 1) * P, :], in_=hn[:])
                # --- l1 normalize branch (f32)
                s1 = wpool.tile([P, 1], f32, tag="s1")
                nc.vector.tensor_reduce(out=s1[:], in_=outl[:],
                                        axis=mybir.AxisListType.X,
                                        op=mybir.AluOpType.add,
                                        apply_absolute_value=True)
                s2 = wpool.tile([P, 1], f32, tag="s2")
                nc.vector.tensor_scalar(out=s2[:], in0=s1[:], scalar1=1e-12,
                                        scalar2=None, op0=mybir.AluOpType.max)
                rs = wpool.tile([P, 1], f32, tag="rs")
                nc.vector.reciprocal(rs[:], s2[:])
                no = wpool.tile([P, FEAT], f32, tag="no")
                nc.vector.tensor_scalar(out=no[:], in0=outl[:],
                                        scalar1=rs[:, 0:1], scalar2=None,
                                        op0=mybir.AluOpType.mult)
                nc.sync.dma_start(out=normout[b * P:(b + 1) * P, :], in_=no[:])
                ioff_b += SIDX
                coff_b += NCH
    nc.compile()
    return nc


# ---------------------------------------------------------------------------
# orchestration
# ---------------------------------------------------------------------------

RUN_INFO = []  # per-launch {name, wall_s, exec_time_ns} (exec only when traced)

_IOTA = None


def _iota_tile():
    global _IOTA
    if _IOTA is None:
        import ml_dtypes
        _IOTA = np.ascontiguousarray(np.broadcast_to(
            np.arange(P, dtype=np.float32).astype(ml_dtypes.bfloat16),
            (P, 8, P)).reshape(P, 8 * P))
    return _IOTA


def _pad_rows(a, n):
    out = np.zeros((n, a.shape[1]), dtype=a.dtype)
    out[:a.shape[0]] = a
    return out


def kernel(x1, x2, edge_index1, edge_index2, W1, b1, W2, b2):
    import ml_dtypes
    from concourse.bass_utils import run_bass_kernel_spmd

    bf16 = ml_dtypes.bfloat16
    x = [np.asarray(x1, np.float32).astype(bf16),
         np.asarray(x2, np.float32).astype(bf16)]
    W1 = np.asarray(W1, np.float32).astype(bf16)
    W2 = np.asarray(W2, np.float32).astype(bf16)
    b1 = np.asarray(b1, np.float32)
    b2 = np.asarray(b2, np.float32)
    cores, wcb = _host_prep(np.asarray(edge_index1), np.asarray(edge_index2))
    core_ids = list(range(N_CORES))

    # ---- launch A: h1' = dis (.) (x @ W1) per shard, bf16
    a_maps = []
    for c in cores:
        g, s = c["graph"], c["shard"]
        xs = _pad_rows(x[g][SHARD * s:SHARD * (s + 1)], SHARD_PAD)
        a_maps.append(dict(xT=np.ascontiguousarray(xs.T), w=W1, disb=c["disb"]))

    import time

    def _run(nc, maps, name):
        t0 = time.time()
        res = run_bass_kernel_spmd(nc, maps, core_ids)
        RUN_INFO.append(dict(name=name, wall_s=time.time() - t0,
                             exec_time_ns=res.exec_time_ns,
                             profile=res.profile_json))
        return res.results

    nc_a = _build_neff_a()
    a_out = _run(nc_a, a_maps, "A")

    def full_table(outs, key):
        tabs = []
        for g in range(2):
            shards = [outs[g * N_SHARDS + s][key][:SHARD] for s in range(N_SHARDS)]
            tabs.append(np.ascontiguousarray(np.concatenate(shards, axis=0)))
        return tabs

    h1 = full_table(a_out, "hout")

    # ---- launch B (x2): aggregation layers
    nc_b = _build_neff_b(wcb)

    def run_b(htabs, bias):
        maps = []
        brep = np.ascontiguousarray(
            np.broadcast_to(bias, (P, FEAT)).astype(np.float32))
        for c in cores:
            g, s = c["graph"], c["shard"]
            maps.append(dict(
                htab=htabs[g],
                hown=_pad_rows(htabs[g][SHARD * s:SHARD * (s + 1)], SHARD_PAD),
                idx16=c["idx16"], dstloc=c["dstloc"], disb=c["disb"],
                brep=brep, iotaf=_iota_tile(), wnext=W2,
            ))
        run_b.n = getattr(run_b, "n", 0) + 1
        return _run(nc_b, maps, f"B{run_b.n}")

    b1_out = run_b(h1, b1)
    h2 = full_table(b1_out, "hnext")
    b2_out = run_b(h2, b2)
    emd = full_table(b2_out, "normout")
    return emd[0], emd[1]


# revision 8
# speedup vs baseline: 3.4747x; 1.4388x over previous
"""Trainium2 Bass kernel for a 2-layer GCN on two graphs (shared weights).

Problem: nn_BRIGHT_gcn (gnn_message_passing).
  reference:
    gcn_conv(x, ei, W, b): deg = 1 + indeg(col); dis = rsqrt(deg)
      h = x @ W; out[c] = sum_{(r,c) in E} dis[r]*dis[c]*h[r] + dis[c]^2*h[c] + b
    two layers, then L1-normalize rows.  Two graphs through the same weights.

Strategy (8 NeuronCores, SPMD):
  - graph g in {0,1} on cores 4g..4g+3; each core owns a contiguous shard of
    25000 destination nodes.
  - Factor the symmetric norm: h' = dis (.) (x @ W) stored as a bf16 table.
    The edge aggregation is a plain segment-sum of h'[src] rows, post-scaled
    by dis[dst]:  out = dis (.) (segsum(h'[src] -> dst) + h'[own]) + b
  - NEFF A: h1' = dis (.) (xT.T @ W1) in bf16 for the core's shard.
  - host: allgather h1' shards -> full bf16 table H' per graph (free).
  - NEFF B (compiled once, run twice): for each 128-dst block, gather the
    incoming edges' bf16 h' rows from HBM with dma_gather (int16 idxs, 4
    windows of 32768 rows, 4 SWDGE queues round-robin so Q7 descriptor
    generation overlaps ~2x) and scatter-add them with one-hot x PE matmuls
    (bf16, 1 cyc/row) into f32 PSUM.  Epilogue computes both
    h_next' = dis (.) ((out+b) @ W2) (bf16, the next layer's table) and
    l1norm(out+b) (f32); the host uses h_next' after layer 1 and l1norm
    after layer 2.
  - chunk counts are per-(block, window) maxima over the 8 cores (not a
    global max), cutting gather padding from +16% to +8%.

kernel() takes FULL inputs and returns the FULL output tuple.
"""

import math

import numpy as np

P = 128
FEAT = 256
N_NODES = 100000
N_CORES = 8
N_SHARDS = 4  # per graph
SHARD = N_NODES // N_SHARDS  # 25000
NBLK = math.ceil(SHARD / P)  # 196
SHARD_PAD = NBLK * P  # 25088
WIN = 32768  # int16 index window
N_WIN = math.ceil(N_NODES / WIN)  # 4
WIN_SIZES = [min(WIN, N_NODES - w * WIN) for w in range(N_WIN)]  # [32768]*3+[1696]
N_QUEUES = 4  # SWDGE descriptor queues (round-robin over gather calls)


# ---------------------------------------------------------------------------
# host-side graph preprocessing
# ---------------------------------------------------------------------------

def _prep_graph(edge_index):
    """Degree vector (with self-loops) for one graph."""
    col = np.asarray(edge_index[1], dtype=np.int64)
    deg = np.bincount(col, minlength=N_NODES).astype(np.float32) + 1.0
    dis = (1.0 / np.sqrt(deg)).astype(np.float32)
    return dis


def _prep_shard_edges(edge_index, shard_id):
    """Bucket one shard's incoming edges by (dst block, src window).

    Returns dict with per-(block, window) counts plus sorted per-edge arrays:
      blk   [e] destination block within shard (0..NBLK-1)
      dloc  [e] destination lane within block (0..127)
      widx  [e] source row within its window (0..32767)
      win   [e] source window (0..3)
      cnt   [NBLK, N_WIN] group sizes
    sorted by (blk, win), stable.
    """
    row = np.asarray(edge_index[0], dtype=np.int64)
    col = np.asarray(edge_index[1], dtype=np.int64)
    lo, hi = SHARD * shard_id, SHARD * (shard_id + 1)
    m = (col >= lo) & (col < hi)
    src = row[m]
    dst = col[m] - lo
    blk = dst >> 7
    dloc = dst & 127
    win = src >> 15
    widx = src & (WIN - 1)
    # widx as the innermost key: ascending row addresses within each gather
    # group make the random reads quasi-sequential in HBM (row-buffer hits)
    order = np.lexsort((widx, win, blk))
    blk, dloc, win, widx = blk[order], dloc[order], win[order], widx[order]
    cnt = np.bincount(blk * N_WIN + win, minlength=NBLK * N_WIN).reshape(NBLK, N_WIN)
    return dict(blk=blk, dloc=dloc, win=win, widx=widx, cnt=cnt)


def _build_core_tables(sh, wcb):
    """Build the per-core device-side index/onehot tables.

    wcb[b, w]: chunks (of 128 edges) allotted to the (block b, window w)
    group — per-(b, w) maxima over cores, shared by the single NEFF.
    Flat free-dim layout per block b:
      idx16 : for w in 0..3: wcb[b,w]*128 int16 window-row indices, wrapped
              [16, L/16] (pos j -> partition j%16, slot j//16), replicated to
              128 partitions.
      dstloc: for w in 0..3: wcb[b,w] columns of 128 bf16 dst lanes (pad=-1).
    Padding edges gather window row 0 and have dstloc -1 (one-hot zero).
    """
    import ml_dtypes

    blk, win, widx, dloc, cnt = sh["blk"], sh["win"], sh["widx"], sh["dloc"], sh["cnt"]
    L = wcb * P  # [NBLK, N_WIN] padded group sizes
    # free-dim offset of group (b, w) in the flat per-edge layout
    flat_sizes = L.reshape(-1)
    gstart = np.zeros(NBLK * N_WIN, dtype=np.int64)
    gstart[1:] = np.cumsum(flat_sizes)[:-1]
    tot = int(flat_sizes.sum())

    flat_cnt = cnt.reshape(-1)
    estart = np.zeros(NBLK * N_WIN, dtype=np.int64)
    estart[1:] = np.cumsum(flat_cnt)[:-1]
    gid = blk * N_WIN + win
    pos_in_group = np.arange(len(blk)) - estart[gid]
    flat_pos = gstart[gid] + pos_in_group

    idx_flat = np.zeros(tot, dtype=np.int16)
    dloc_flat = np.full(tot, -1.0, dtype=np.float32)
    idx_flat[flat_pos] = widx.astype(np.int16)
    dloc_flat[flat_pos] = dloc.astype(np.float32)

    # idx16: per (b, w) wrap [L] -> [16, L/16]; concat along free dim;
    # replicate to 128 partitions.
    idx_parts = []
    dl_parts = []
    for b in range(NBLK):
        for w in range(N_WIN):
            g0 = gstart[b * N_WIN + w]
            seg = idx_flat[g0:g0 + L[b, w]]
            idx_parts.append(seg.reshape(L[b, w] // 16, 16).T)  # [16, L/16]
            dl_parts.append(dloc_flat[g0:g0 + L[b, w]].reshape(wcb[b, w], P).T)
    idx16 = np.concatenate(idx_parts, axis=1)  # [16, tot/16]
    idx16 = np.tile(idx16, (8, 1))  # [128, tot/16]
    dstloc = np.concatenate(dl_parts, axis=1).astype(ml_dtypes.bfloat16)
    return (np.ascontiguousarray(idx16), np.ascontiguousarray(dstloc))


def _dis_cols(dis, shard_id):
    """dis for the shard as [128, NBLK] (partition = lane in block)."""
    d = np.zeros(SHARD_PAD, dtype=np.float32)
    d[:SHARD] = dis[SHARD * shard_id:SHARD * (shard_id + 1)]
    return np.ascontiguousarray(d.reshape(NBLK, P).T)


def _host_prep(edge_index1, edge_index2):
    """All static per-core structures. Returns (cores, wcb)."""
    dis = [_prep_graph(edge_index1), _prep_graph(edge_index2)]
    shards = []
    for g, ei in enumerate((edge_index1, edge_index2)):
        for s in range(N_SHARDS):
            shards.append((g, s, _prep_shard_edges(ei, s)))
    # per-(block, window) chunk counts: max over the 8 cores
    cnt_max = np.stack([sh["cnt"] for _, _, sh in shards]).max(axis=0)
    wcb = np.maximum(1, np.ceil(cnt_max / P).astype(np.int64))  # [NBLK, N_WIN]
    cnt16 = np.maximum(16, np.ceil(cnt_max / 16).astype(np.int64) * 16)
    cores = []
    for g, s, sh in shards:
        idx16, dstloc = _build_core_tables(sh, wcb)
        cores.append(dict(
            graph=g, shard=s,
            idx16=idx16, dstloc=dstloc,
            disb=_dis_cols(dis[g], s),
            dis=dis[g],
        ))
    return cores, wcb, cnt16


# ---------------------------------------------------------------------------
# device kernels (bass/tile)
# ---------------------------------------------------------------------------

def _build_neff_a():
    import concourse.bacc as bacc
    import concourse.mybir as mybir
    import concourse.tile as tile

    f32 = mybir.dt.float32
    bf16 = mybir.dt.bfloat16
    nc = bacc.Bacc("TRN2", target_bir_lowering=False, debug=False)
    xT = nc.dram_tensor("xT", [FEAT, SHARD_PAD], bf16, kind="ExternalInput")
    w_in = nc.dram_tensor("w", [FEAT, FEAT], bf16, kind="ExternalInput")
    disb = nc.dram_tensor("disb", [P, NBLK], f32, kind="ExternalInput")
    hout = nc.dram_tensor("hout", [SHARD_PAD, FEAT], bf16, kind="ExternalOutput")

    with tile.TileContext(nc) as tc:
        with (
            tc.tile_pool(name="const", bufs=1) as cpool,
            tc.tile_pool(name="work", bufs=3) as wpool,
            tc.tile_pool(name="psum", bufs=2, space="PSUM") as ppool,
        ):
            w_sb = cpool.tile([P, 2, FEAT], bf16, tag="w")
            nc.sync.dma_start(out=w_sb[:, 0, :], in_=w_in[0:P, :])
            nc.sync.dma_start(out=w_sb[:, 1, :], in_=w_in[P:FEAT, :])
            dis_sb = cpool.tile([P, NBLK], f32, tag="dis")
            nc.sync.dma_start(out=dis_sb[:], in_=disb[:, :])
            for t in range(NBLK):
                xt = wpool.tile([P, 2, P], bf16, tag="xT")
                nc.sync.dma_start(out=xt[:, 0, :], in_=xT[0:P, t * P:(t + 1) * P])
                nc.sync.dma_start(out=xt[:, 1, :], in_=xT[P:FEAT, t * P:(t + 1) * P])
                ps = ppool.tile([P, FEAT], f32, tag="ps")
                nc.tensor.matmul(ps[:], lhsT=xt[:, 0, :], rhs=w_sb[:, 0, :],
                                 start=True, stop=False)
                nc.tensor.matmul(ps[:], lhsT=xt[:, 1, :], rhs=w_sb[:, 1, :],
                                 start=False, stop=True)
                hs = wpool.tile([P, FEAT], bf16, tag="hs")
                nc.vector.tensor_scalar(out=hs[:], in0=ps[:],
                                        scalar1=dis_sb[:, t:t + 1], scalar2=None,
                                        op0=mybir.AluOpType.mult)
                nc.sync.dma_start(out=hout[t * P:(t + 1) * P, :], in_=hs[:])
    nc.compile()
    return nc


def _build_neff_b(wcb, cnt16):
    import concourse.bacc as bacc
    import concourse.mybir as mybir
    import concourse.tile as tile
    from concourse.masks import make_identity

    f32 = mybir.dt.float32
    bf16 = mybir.dt.bfloat16
    i16 = mybir.dt.int16
    SIDXb = [int(wcb[b].sum()) * 8 for b in range(NBLK)]  # int16 cols per block
    NCHb = [int(wcb[b].sum()) for b in range(NBLK)]  # chunks per block
    TOT_SIDX = sum(SIDXb)
    TOT_NCH = sum(NCHb)
    SIDX_MAX = max(SIDXb)
    NCH_MAX = max(NCHb)
    nc = bacc.Bacc("TRN2", target_bir_lowering=False, debug=False,
                   num_swdge_queues=N_QUEUES)
    htab = nc.dram_tensor("htab", [N_NODES, FEAT], bf16, kind="ExternalInput")
    # hot2 = dis (.) h'own + b  (self-loop term + bias, host-precomputed)
    hot2 = nc.dram_tensor("hot2", [SHARD_PAD, FEAT], bf16, kind="ExternalInput")
    idx16 = nc.dram_tensor("idx16", [P, TOT_SIDX], i16, kind="ExternalInput")
    dstloc = nc.dram_tensor("dstloc", [P, TOT_NCH], bf16, kind="ExternalInput")
    disb = nc.dram_tensor("disb", [P, NBLK], f32, kind="ExternalInput")
    OHG = 16  # one-hot chunks generated per DVE op
    iotaf = nc.dram_tensor("iotaf", [P, OHG * P], bf16, kind="ExternalInput")
    wnext = nc.dram_tensor("wnext", [FEAT, FEAT], bf16, kind="ExternalInput")
    hnext = nc.dram_tensor("hnext", [SHARD_PAD, FEAT], bf16,
                           kind="ExternalOutput")
    normout = nc.dram_tensor("normout", [SHARD_PAD, FEAT], f32,
                             kind="ExternalOutput")

    qc = [0]  # SWDGE queue rotation counter

    def next_q():
        q = qc[0] % N_QUEUES
        qc[0] += 1
        return q

    with tile.TileContext(nc) as tc:
        with (
            tc.tile_pool(name="const", bufs=1) as cpool,
            tc.tile_pool(name="gland", bufs=2) as gpool,
            tc.tile_pool(name="work", bufs=3) as wpool,
            tc.tile_pool(name="oh", bufs=6) as ohpool,
            tc.tile_pool(name="psum", bufs=2, space="PSUM") as ppool,
            tc.tile_pool(name="psumt", bufs=2, space="PSUM") as ptpool,
        ):
            w_sb = cpool.tile([P, 2, FEAT], bf16, tag="w")
            nc.sync.dma_start(out=w_sb[:, 0, :], in_=wnext[0:P, :])
            nc.sync.dma_start(out=w_sb[:, 1, :], in_=wnext[P:FEAT, :])
            dis_sb = cpool.tile([P, NBLK], f32, tag="dis")
            nc.sync.dma_start(out=dis_sb[:], in_=disb[:, :])
            iota_sb = cpool.tile([P, OHG, P], bf16, tag="iota")
            nc.sync.dma_start(out=iota_sb[:], in_=iotaf[:, :].rearrange(
                "p (g q) -> p g q", g=OHG))
            dl_sb = cpool.tile([P, TOT_NCH], bf16, tag="dl")
            nc.sync.dma_start(out=dl_sb[:], in_=dstloc[:, :])
            ident = cpool.tile([P, P], bf16, tag="ident")
            make_identity(nc, ident[:])

            ioff_b = 0  # running idx16 free-dim offset
            coff_b = 0  # running dstloc chunk-col offset
            for b in range(NBLK):
                SIDX = SIDXb[b]
                NCH = NCHb[b]
                idxt = wpool.tile([P, SIDX_MAX], i16, tag="idx")
                nc.sync.dma_start(out=idxt[:, 0:SIDX],
                                  in_=idx16[:, ioff_b:ioff_b + SIDX])
                gt = gpool.tile([P, NCH_MAX, FEAT], bf16, tag="g")
                ioff_w = 0
                goff = 0
                for w in range(N_WIN):
                    wc = int(wcb[b, w])
                    n16 = int(cnt16[b, w])  # 16-padded max edge count
                    src = htab[w * WIN:w * WIN + WIN_SIZES[w], :]
                    for p0 in range(0, wc, 8):
                        pc = min(8, wc - p0)
                        nq_rows = min(pc * P, n16 - p0 * P)
                        nc.gpsimd.dma_gather(
                            gt[:, goff + p0:goff + p0 + pc, :],
                            src,
                            idxt[:, ioff_w + p0 * 8:
                                 ioff_w + p0 * 8 + nq_rows // 16],
                            nq_rows,
                            nq_rows,
                            FEAT,
                            single_packet=False,
                            queue_num=next_q(),
                        )
                    ioff_w += wc * 8
                    goff += wc
                ps = ppool.tile([P, FEAT], f32, tag="agg")
                # batched one-hot generation: OHG chunks per DVE op
                ohs = []
                for c0 in range(0, NCH, OHG):
                    g = min(OHG, NCH - c0)
                    oh = ohpool.tile([P, OHG, P], bf16, tag="oh")
                    nc.vector.tensor_tensor(
                        out=oh[:, :g, :], in0=iota_sb[:, :g, :],
                        in1=dl_sb[:, coff_b + c0:coff_b + c0 + g]
                        .to_broadcast([P, g, P]),
                        op=mybir.AluOpType.is_equal)
                    ohs.append(oh)
                for j in range(NCH):
                    oh = ohs[j // OHG][:, j % OHG, :]
                    nc.tensor.matmul(ps[:], lhsT=oh, rhs=gt[:, j, :],
                                     start=(j == 0), stop=(j == NCH - 1))
                hot = wpool.tile([P, FEAT], bf16, tag="hot")
                nc.sync.dma_start(out=hot[:], in_=hot2[b * P:(b + 1) * P, :])
                # t2 = dis (.) ps on the (otherwise idle) scalar engine
                t2 = wpool.tile([P, FEAT], f32, tag="t2")
                nc.scalar.activation(out=t2[:], in_=ps[:],
                                     func=mybir.ActivationFunctionType.Copy,
                                     scale=dis_sb[:, b:b + 1])
                # ob = layer output in bf16 (= t2 + hot2)
                ob = wpool.tile([P, FEAT], bf16, tag="ob")
                nc.vector.tensor_tensor(out=ob[:], in0=t2[:], in1=hot[:],
                                        op=mybir.AluOpType.add)
                # --- h_next branch: dis (.) (ob @ W2), bf16
                tp = ptpool.tile([P, 2, P], bf16, tag="tp")
                nc.tensor.transpose(tp[:, 0, :], ob[:, 0:P], ident[:])
                nc.tensor.transpose(tp[:, 1, :], ob[:, P:FEAT], ident[:])
                tts = wpool.tile([P, 2, P], bf16, tag="tts")
                nc.vector.tensor_copy(tts[:, 0, :], tp[:, 0, :])
                nc.scalar.copy(tts[:, 1, :], tp[:, 1, :])
                ps2 = ppool.tile([P, FEAT], f32, tag="mm2")
                nc.tensor.matmul(ps2[:], lhsT=tts[:, 0, :], rhs=w_sb[:, 0, :],
                                 start=True, stop=False)
                nc.tensor.matmul(ps2[:], lhsT=tts[:, 1, :], rhs=w_sb[:, 1, :],
                                 start=False, stop=True)
                hn = wpool.tile([P, FEAT], bf16, tag="hn")
                nc.scalar.activation(out=hn[:], in_=ps2[:],
                                     func=mybir.ActivationFunctionType.Copy,
                                     scale=dis_sb[:, b:b + 1])
                nc.sync.dma_start(out=hnext[b * P:(b + 1) * P, :], in_=hn[:])
                # --- l1 normalize branch
                s1 = wpool.tile([P, 1], f32, tag="s1")
                nc.vector.tensor_reduce(out=s1[:], in_=ob[:],
                                        axis=mybir.AxisListType.X,
                                        op=mybir.AluOpType.add,
                                        apply_absolute_value=True)
                s2 = wpool.tile([P, 1], f32, tag="s2")
                nc.vector.tensor_scalar(out=s2[:], in0=s1[:], scalar1=1e-12,
                                        scalar2=None, op0=mybir.AluOpType.max)
                rs = wpool.tile([P, 1], f32, tag="rs")
                nc.vector.reciprocal(rs[:], s2[:])
                no = wpool.tile([P, FEAT], f32, tag="no")
                nc.scalar.activation(out=no[:], in_=ob[:],
                                     func=mybir.ActivationFunctionType.Copy,
                                     scale=rs[:, 0:1])
                nc.sync.dma_start(out=normout[b * P:(b + 1) * P, :], in_=no[:])
                ioff_b += SIDX
                coff_b += NCH
    nc.compile()
    return nc


# ---------------------------------------------------------------------------
# orchestration
# ---------------------------------------------------------------------------

RUN_INFO = []  # per-launch {name, wall_s, exec_time_ns} (exec only when traced)

_IOTA = None


def _iota_tile():
    global _IOTA
    if _IOTA is None:
        import ml_dtypes
        _IOTA = np.ascontiguousarray(np.broadcast_to(
            np.arange(P, dtype=np.float32).astype(ml_dtypes.bfloat16),
            (P, 16, P)).reshape(P, 16 * P))
    return _IOTA


def _pad_rows(a, n):
    out = np.zeros((n, a.shape[1]), dtype=a.dtype)
    out[:a.shape[0]] = a
    return out


def kernel(x1, x2, edge_index1, edge_index2, W1, b1, W2, b2):
    import ml_dtypes
    from concourse.bass_utils import run_bass_kernel_spmd

    bf16 = ml_dtypes.bfloat16
    x = [np.asarray(x1, np.float32).astype(bf16),
         np.asarray(x2, np.float32).astype(bf16)]
    W1 = np.asarray(W1, np.float32).astype(bf16)
    W2 = np.asarray(W2, np.float32).astype(bf16)
    b1 = np.asarray(b1, np.float32)
    b2 = np.asarray(b2, np.float32)
    cores, wcb, cnt16 = _host_prep(np.asarray(edge_index1), np.asarray(edge_index2))
    core_ids = list(range(N_CORES))

    # ---- launch A: h1' = dis (.) (x @ W1) per shard, bf16
    a_maps = []
    for c in cores:
        g, s = c["graph"], c["shard"]
        xs = _pad_rows(x[g][SHARD * s:SHARD * (s + 1)], SHARD_PAD)
        a_maps.append(dict(xT=np.ascontiguousarray(xs.T), w=W1, disb=c["disb"]))

    import time

    def _run(nc, maps, name):
        t0 = time.time()
        res = run_bass_kernel_spmd(nc, maps, core_ids)
        RUN_INFO.append(dict(name=name, wall_s=time.time() - t0,
                             exec_time_ns=res.exec_time_ns,
                             profile=res.profile_json))
        return res.results

    nc_a = _build_neff_a()
    a_out = _run(nc_a, a_maps, "A")

    def full_table(outs, key):
        tabs = []
        for g in range(2):
            shards = [outs[g * N_SHARDS + s][key][:SHARD] for s in range(N_SHARDS)]
            tabs.append(np.ascontiguousarray(np.concatenate(shards, axis=0)))
        return tabs

    h1 = full_table(a_out, "hout")

    # ---- launch B (x2): aggregation layers
    nc_b = _build_neff_b(wcb, cnt16)

    def run_b(htabs, bias):
        maps = []
        for c in cores:
            g, s = c["graph"], c["shard"]
            dis_sh = _pad_rows(
                c["dis"][SHARD * s:SHARD * (s + 1), None].astype(np.float32),
                SHARD_PAD)
            own = _pad_rows(htabs[g][SHARD * s:SHARD * (s + 1)], SHARD_PAD)
            h2v = (own.astype(np.float32) * dis_sh +
                   bias[None, :]).astype(own.dtype)
            maps.append(dict(
                htab=htabs[g],
                hot2=h2v,
                idx16=c["idx16"], dstloc=c["dstloc"], disb=c["disb"],
                iotaf=_iota_tile(), wnext=W2,
            ))
        run_b.n = getattr(run_b, "n", 0) + 1
        return _run(nc_b, maps, f"B{run_b.n}")

    b1_out = run_b(h1, b1)
    h2 = full_table(b1_out, "hnext")
    b2_out = run_b(h2, b2)
    emd = full_table(b2_out, "normout")
    return emd[0], emd[1]


# revision 11
# speedup vs baseline: 3.4874x; 1.0037x over previous
"""Trainium2 Bass kernel for a 2-layer GCN on two graphs (shared weights).

Problem: nn_BRIGHT_gcn (gnn_message_passing).
  reference:
    gcn_conv(x, ei, W, b): deg = 1 + indeg(col); dis = rsqrt(deg)
      h = x @ W; out[c] = sum_{(r,c) in E} dis[r]*dis[c]*h[r] + dis[c]^2*h[c] + b
    two layers, then L1-normalize rows.  Two graphs through the same weights.

Strategy (8 NeuronCores, SPMD):
  - graph g in {0,1} on cores 4g..4g+3; each core owns a contiguous shard of
    25000 destination nodes.
  - Factor the symmetric norm: h' = dis (.) (x @ W) stored as a bf16 table.
    The edge aggregation is a plain segment-sum of h'[src] rows, post-scaled
    by dis[dst]:  out = dis (.) (segsum(h'[src] -> dst) + h'[own]) + b
  - NEFF A: h1' = dis (.) (xT.T @ W1) in bf16 for the core's shard.
  - host: allgather h1' shards -> full bf16 table H' per graph (free).
  - NEFF B (compiled once, run twice): for each 128-dst block, gather the
    incoming edges' bf16 h' rows from HBM with dma_gather (int16 idxs, 4
    windows of 32768 rows, 4 SWDGE queues round-robin so Q7 descriptor
    generation overlaps ~2x) and scatter-add them with one-hot x PE matmuls
    (bf16, 1 cyc/row) into f32 PSUM.  Epilogue computes both
    h_next' = dis (.) ((out+b) @ W2) (bf16, the next layer's table) and
    l1norm(out+b) (f32); the host uses h_next' after layer 1 and l1norm
    after layer 2.
  - chunk counts are per-(block, window) maxima over the 8 cores (not a
    global max), cutting gather padding from +16% to +8%.

kernel() takes FULL inputs and returns the FULL output tuple.
"""

import math

import numpy as np

P = 128
FEAT = 256
N_NODES = 100000
N_CORES = 8
N_SHARDS = 4  # per graph
SHARD = N_NODES // N_SHARDS  # 25000
NBLK = math.ceil(SHARD / P)  # 196
SHARD_PAD = NBLK * P  # 25088
WIN = 32768  # int16 index window
N_WIN = math.ceil(N_NODES / WIN)  # 4
WIN_SIZES = [min(WIN, N_NODES - w * WIN) for w in range(N_WIN)]  # [32768]*3+[1696]
N_QUEUES = 4  # SWDGE descriptor queues (round-robin over gather calls)


# ---------------------------------------------------------------------------
# host-side graph preprocessing
# ---------------------------------------------------------------------------

def _prep_graph(edge_index):
    """Degree vector (with self-loops) for one graph."""
    col = np.asarray(edge_index[1], dtype=np.int64)
    deg = np.bincount(col, minlength=N_NODES).astype(np.float32) + 1.0
    dis = (1.0 / np.sqrt(deg)).astype(np.float32)
    return dis


def _prep_shard_edges(edge_index, shard_id):
    """Bucket one shard's incoming edges by (dst block, src window).

    Returns dict with per-(block, window) counts plus sorted per-edge arrays:
      blk   [e] destination block within shard (0..NBLK-1)
      dloc  [e] destination lane within block (0..127)
      widx  [e] source row within its window (0..32767)
      win   [e] source window (0..3)
      cnt   [NBLK, N_WIN] group sizes
    sorted by (blk, win), stable.
    """
    row = np.asarray(edge_index[0], dtype=np.int64)
    col = np.asarray(edge_index[1], dtype=np.int64)
    lo, hi = SHARD * shard_id, SHARD * (shard_id + 1)
    m = (col >= lo) & (col < hi)
    src = row[m]
    dst = col[m] - lo
    blk = dst >> 7
    dloc = dst & 127
    win = src >> 15
    widx = src & (WIN - 1)
    # widx as the innermost key: ascending row addresses within each gather
    # group make the random reads quasi-sequential in HBM (row-buffer hits)
    order = np.lexsort((widx, win, blk))
    blk, dloc, win, widx = blk[order], dloc[order], win[order], widx[order]
    cnt = np.bincount(blk * N_WIN + win, minlength=NBLK * N_WIN).reshape(NBLK, N_WIN)
    return dict(blk=blk, dloc=dloc, win=win, widx=widx, cnt=cnt)


def _build_core_tables(sh, wcb):
    """Build the per-core device-side index/onehot tables.

    wcb[b, w]: chunks (of 128 edges) allotted to the (block b, window w)
    group — per-(b, w) maxima over cores, shared by the single NEFF.
    Flat free-dim layout per block b:
      idx16 : for w in 0..3: wcb[b,w]*128 int16 window-row indices, wrapped
              [16, L/16] (pos j -> partition j%16, slot j//16), replicated to
              128 partitions.
      dstloc: for w in 0..3: wcb[b,w] columns of 128 bf16 dst lanes (pad=-1).
    Padding edges gather window row 0 and have dstloc -1 (one-hot zero).
    """
    import ml_dtypes

    blk, win, widx, dloc, cnt = sh["blk"], sh["win"], sh["widx"], sh["dloc"], sh["cnt"]
    L = wcb * P  # [NBLK, N_WIN] padded group sizes
    # free-dim offset of group (b, w) in the flat per-edge layout
    flat_sizes = L.reshape(-1)
    gstart = np.zeros(NBLK * N_WIN, dtype=np.int64)
    gstart[1:] = np.cumsum(flat_sizes)[:-1]
    tot = int(flat_sizes.sum())

    flat_cnt = cnt.reshape(-1)
    estart = np.zeros(NBLK * N_WIN, dtype=np.int64)
    estart[1:] = np.cumsum(flat_cnt)[:-1]
    gid = blk * N_WIN + win
    pos_in_group = np.arange(len(blk)) - estart[gid]
    flat_pos = gstart[gid] + pos_in_group

    idx_flat = np.zeros(tot, dtype=np.int16)
    dloc_flat = np.full(tot, -1.0, dtype=np.float32)
    idx_flat[flat_pos] = widx.astype(np.int16)
    dloc_flat[flat_pos] = dloc.astype(np.float32)

    # idx16: per (b, w) wrap [L] -> [16, L/16]; concat along free dim;
    # replicate to 128 partitions.
    idx_parts = []
    dl_parts = []
    for b in range(NBLK):
        for w in range(N_WIN):
            g0 = gstart[b * N_WIN + w]
            seg = idx_flat[g0:g0 + L[b, w]]
            idx_parts.append(seg.reshape(L[b, w] // 16, 16).T)  # [16, L/16]
            dl_parts.append(dloc_flat[g0:g0 + L[b, w]].reshape(wcb[b, w], P).T)
    idx16 = np.concatenate(idx_parts, axis=1)  # [16, tot/16]
    idx16 = np.tile(idx16, (8, 1))  # [128, tot/16]
    dstloc = np.concatenate(dl_parts, axis=1).astype(ml_dtypes.bfloat16)
    return (np.ascontiguousarray(idx16), np.ascontiguousarray(dstloc))


def _dis_cols(dis, shard_id):
    """dis for the shard as [128, NBLK] (partition = lane in block)."""
    d = np.zeros(SHARD_PAD, dtype=np.float32)
    d[:SHARD] = dis[SHARD * shard_id:SHARD * (shard_id + 1)]
    return np.ascontiguousarray(d.reshape(NBLK, P).T)


def _host_prep(edge_index1, edge_index2):
    """All static per-core structures. Returns (cores, wcb)."""
    dis = [_prep_graph(edge_index1), _prep_graph(edge_index2)]
    shards = []
    for g, ei in enumerate((edge_index1, edge_index2)):
        for s in range(N_SHARDS):
            shards.append((g, s, _prep_shard_edges(ei, s)))
    # per-(block, window) chunk counts: max over the 8 cores
    cnt_max = np.stack([sh["cnt"] for _, _, sh in shards]).max(axis=0)
    wcb = np.maximum(1, np.ceil(cnt_max / P).astype(np.int64))  # [NBLK, N_WIN]
    cnt16 = np.maximum(16, np.ceil(cnt_max / 16).astype(np.int64) * 16)
    cores = []
    for g, s, sh in shards:
        idx16, dstloc = _build_core_tables(sh, wcb)
        cores.append(dict(
            graph=g, shard=s,
            idx16=idx16, dstloc=dstloc,
            disb=_dis_cols(dis[g], s),
            dis=dis[g],
        ))
    return cores, wcb, cnt16


# ---------------------------------------------------------------------------
# device kernels (bass/tile)
# ---------------------------------------------------------------------------

def _build_neff_a():
    import concourse.bacc as bacc
    import concourse.mybir as mybir
    import concourse.tile as tile

    f32 = mybir.dt.float32
    bf16 = mybir.dt.bfloat16
    nc = bacc.Bacc("TRN2", target_bir_lowering=False, debug=False)
    xT = nc.dram_tensor("xT", [FEAT, SHARD_PAD], bf16, kind="ExternalInput")
    w_in = nc.dram_tensor("w", [FEAT, FEAT], bf16, kind="ExternalInput")
    disb = nc.dram_tensor("disb", [P, NBLK], f32, kind="ExternalInput")
    hout = nc.dram_tensor("hout", [SHARD_PAD, FEAT], bf16, kind="ExternalOutput")

    with tile.TileContext(nc) as tc:
        with (
            tc.tile_pool(name="const", bufs=1) as cpool,
            tc.tile_pool(name="work", bufs=3) as wpool,
            tc.tile_pool(name="psum", bufs=2, space="PSUM") as ppool,
        ):
            w_sb = cpool.tile([P, 2, FEAT], bf16, tag="w")
            nc.sync.dma_start(out=w_sb[:, 0, :], in_=w_in[0:P, :])
            nc.sync.dma_start(out=w_sb[:, 1, :], in_=w_in[P:FEAT, :])
            dis_sb = cpool.tile([P, NBLK], f32, tag="dis")
            nc.sync.dma_start(out=dis_sb[:], in_=disb[:, :])
            for t in range(NBLK):
                xt = wpool.tile([P, 2, P], bf16, tag="xT")
                nc.sync.dma_start(out=xt[:, 0, :], in_=xT[0:P, t * P:(t + 1) * P])
                nc.sync.dma_start(out=xt[:, 1, :], in_=xT[P:FEAT, t * P:(t + 1) * P])
                ps = ppool.tile([P, FEAT], f32, tag="ps")
                nc.tensor.matmul(ps[:], lhsT=xt[:, 0, :], rhs=w_sb[:, 0, :],
                                 start=True, stop=False)
                nc.tensor.matmul(ps[:], lhsT=xt[:, 1, :], rhs=w_sb[:, 1, :],
                                 start=False, stop=True)
                # dis is folded into xT on the host: h' = (dis (.) x) @ W
                hs = wpool.tile([P, FEAT], bf16, tag="hs")
                nc.vector.tensor_scalar(out=hs[:], in0=ps[:], scalar1=1.0,
                                        scalar2=None, op0=mybir.AluOpType.mult)
                nc.sync.dma_start(out=hout[t * P:(t + 1) * P, :], in_=hs[:])
    nc.compile()
    return nc


def _build_neff_b(wcb, cnt16):
    import concourse.bacc as bacc
    import concourse.mybir as mybir
    import concourse.tile as tile
    from concourse.masks import make_identity

    f32 = mybir.dt.float32
    bf16 = mybir.dt.bfloat16
    i16 = mybir.dt.int16
    SIDXb = [int(wcb[b].sum()) * 8 for b in range(NBLK)]  # int16 cols per block
    NCHb = [int(wcb[b].sum()) for b in range(NBLK)]  # chunks per block
    TOT_SIDX = sum(SIDXb)
    TOT_NCH = sum(NCHb)
    SIDX_MAX = max(SIDXb)
    NCH_MAX = max(NCHb)
    nc = bacc.Bacc("TRN2", target_bir_lowering=False, debug=False,
                   num_swdge_queues=N_QUEUES)
    htab = nc.dram_tensor("htab", [N_NODES, FEAT], bf16, kind="ExternalInput")
    # hot2 = dis (.) h'own + b  (self-loop term + bias, host-precomputed)
    hot2 = nc.dram_tensor("hot2", [SHARD_PAD, FEAT], bf16, kind="ExternalInput")
    idx16 = nc.dram_tensor("idx16", [P, TOT_SIDX], i16, kind="ExternalInput")
    dstloc = nc.dram_tensor("dstloc", [P, TOT_NCH], bf16, kind="ExternalInput")
    disb = nc.dram_tensor("disb", [P, NBLK], f32, kind="ExternalInput")
    OHG = 16  # one-hot chunks generated per DVE op
    iotaf = nc.dram_tensor("iotaf", [P, OHG * P], bf16, kind="ExternalInput")
    wnext = nc.dram_tensor("wnext", [FEAT, FEAT], bf16, kind="ExternalInput")
    hnext = nc.dram_tensor("hnext", [SHARD_PAD, FEAT], bf16,
                           kind="ExternalOutput")
    normout = nc.dram_tensor("normout", [SHARD_PAD, FEAT], f32,
                             kind="ExternalOutput")

    qc = [0]  # SWDGE queue rotation counter

    def next_q():
        q = qc[0] % N_QUEUES
        qc[0] += 1
        return q

    with tile.TileContext(nc) as tc:
        with (
            tc.tile_pool(name="const", bufs=1) as cpool,
            tc.tile_pool(name="gland", bufs=2) as gpool,
            tc.tile_pool(name="work", bufs=3) as wpool,
            tc.tile_pool(name="oh", bufs=6) as ohpool,
            tc.tile_pool(name="psum", bufs=2, space="PSUM") as ppool,
            tc.tile_pool(name="psumt", bufs=2, space="PSUM") as ptpool,
        ):
            w_sb = cpool.tile([P, 2, FEAT], bf16, tag="w")
            nc.sync.dma_start(out=w_sb[:, 0, :], in_=wnext[0:P, :])
            nc.sync.dma_start(out=w_sb[:, 1, :], in_=wnext[P:FEAT, :])
            dis_sb = cpool.tile([P, NBLK], f32, tag="dis")
            nc.sync.dma_start(out=dis_sb[:], in_=disb[:, :])
            iota_sb = cpool.tile([P, OHG, P], bf16, tag="iota")
            nc.sync.dma_start(out=iota_sb[:], in_=iotaf[:, :].rearrange(
                "p (g q) -> p g q", g=OHG))
            dl_sb = cpool.tile([P, TOT_NCH], bf16, tag="dl")
            nc.sync.dma_start(out=dl_sb[:], in_=dstloc[:, :])
            ident = cpool.tile([P, P], bf16, tag="ident")
            make_identity(nc, ident[:])

            ioff_b = 0  # running idx16 free-dim offset
            coff_b = 0  # running dstloc chunk-col offset
            for b in range(NBLK):
                SIDX = SIDXb[b]
                NCH = NCHb[b]
                idxt = wpool.tile([P, SIDX_MAX], i16, tag="idx")
                nc.sync.dma_start(out=idxt[:, 0:SIDX],
                                  in_=idx16[:, ioff_b:ioff_b + SIDX])
                gt = gpool.tile([P, NCH_MAX, FEAT], bf16, tag="g")
                ioff_w = 0
                goff = 0
                for w in range(N_WIN):
                    wc = int(wcb[b, w])
                    n16 = int(cnt16[b, w])  # 16-padded max edge count
                    src = htab[w * WIN:w * WIN + WIN_SIZES[w], :]
                    for p0 in range(0, wc, 8):
                        pc = min(8, wc - p0)
                        nq_rows = min(pc * P, n16 - p0 * P)
                        nc.gpsimd.dma_gather(
                            gt[:, goff + p0:goff + p0 + pc, :],
                            src,
                            idxt[:, ioff_w + p0 * 8:
                                 ioff_w + p0 * 8 + nq_rows // 16],
                            nq_rows,
                            nq_rows,
                            FEAT,
                            single_packet=False,
                            queue_num=next_q(),
                        )
                    ioff_w += wc * 8
                    goff += wc
                ps = ppool.tile([P, FEAT], f32, tag="agg")
                # batched one-hot generation: OHG chunks per DVE op
                ohs = []
                for c0 in range(0, NCH, OHG):
                    g = min(OHG, NCH - c0)
                    oh = ohpool.tile([P, OHG, P], bf16, tag="oh")
                    nc.vector.tensor_tensor(
                        out=oh[:, :g, :], in0=iota_sb[:, :g, :],
                        in1=dl_sb[:, coff_b + c0:coff_b + c0 + g]
                        .to_broadcast([P, g, P]),
                        op=mybir.AluOpType.is_equal)
                    ohs.append(oh)
                for j in range(NCH):
                    oh = ohs[j // OHG][:, j % OHG, :]
                    nc.tensor.matmul(ps[:], lhsT=oh, rhs=gt[:, j, :],
                                     start=(j == 0), stop=(j == NCH - 1))
                hot = wpool.tile([P, FEAT], bf16, tag="hot")
                nc.sync.dma_start(out=hot[:], in_=hot2[b * P:(b + 1) * P, :])
                # t2 = dis (.) ps on the (otherwise idle) scalar engine
                t2 = wpool.tile([P, FEAT], f32, tag="t2")
                nc.scalar.activation(out=t2[:], in_=ps[:],
                                     func=mybir.ActivationFunctionType.Copy,
                                     scale=dis_sb[:, b:b + 1])
                # ob = layer output in bf16 (= t2 + hot2)
                ob = wpool.tile([P, FEAT], bf16, tag="ob")
                nc.vector.tensor_tensor(out=ob[:], in0=t2[:], in1=hot[:],
                                        op=mybir.AluOpType.add)
                # --- h_next branch: dis (.) (ob @ W2), bf16
                tp = ptpool.tile([P, 2, P], bf16, tag="tp")
                nc.tensor.transpose(tp[:, 0, :], ob[:, 0:P], ident[:])
                nc.tensor.transpose(tp[:, 1, :], ob[:, P:FEAT], ident[:])
                tts = wpool.tile([P, 2, P], bf16, tag="tts")
                nc.vector.tensor_copy(tts[:, 0, :], tp[:, 0, :])
                nc.scalar.copy(tts[:, 1, :], tp[:, 1, :])
                ps2 = ppool.tile([P, FEAT], f32, tag="mm2")
                nc.tensor.matmul(ps2[:], lhsT=tts[:, 0, :], rhs=w_sb[:, 0, :],
                                 start=True, stop=False)
                nc.tensor.matmul(ps2[:], lhsT=tts[:, 1, :], rhs=w_sb[:, 1, :],
                                 start=False, stop=True)
                hn = wpool.tile([P, FEAT], bf16, tag="hn")
                nc.scalar.activation(out=hn[:], in_=ps2[:],
                                     func=mybir.ActivationFunctionType.Copy,
                                     scale=dis_sb[:, b:b + 1])
                nc.sync.dma_start(out=hnext[b * P:(b + 1) * P, :], in_=hn[:])
                # --- l1 normalize branch
                s1 = wpool.tile([P, 1], f32, tag="s1")
                nc.vector.tensor_reduce(out=s1[:], in_=ob[:],
                                        axis=mybir.AxisListType.X,
                                        op=mybir.AluOpType.add,
                                        apply_absolute_value=True)
                s2 = wpool.tile([P, 1], f32, tag="s2")
                nc.vector.tensor_scalar(out=s2[:], in0=s1[:], scalar1=1e-12,
                                        scalar2=None, op0=mybir.AluOpType.max)
                rs = wpool.tile([P, 1], f32, tag="rs")
                nc.vector.reciprocal(rs[:], s2[:])
                no = wpool.tile([P, FEAT], f32, tag="no")
                nc.scalar.activation(out=no[:], in_=ob[:],
                                     func=mybir.ActivationFunctionType.Copy,
                                     scale=rs[:, 0:1])
                nc.sync.dma_start(out=normout[b * P:(b + 1) * P, :], in_=no[:])
                ioff_b += SIDX
                coff_b += NCH
    nc.compile()
    return nc


# ---------------------------------------------------------------------------
# orchestration
# ---------------------------------------------------------------------------

RUN_INFO = []  # per-launch {name, wall_s, exec_time_ns} (exec only when traced)

_IOTA = None


def _iota_tile():
    global _IOTA
    if _IOTA is None:
        import ml_dtypes
        _IOTA = np.ascontiguousarray(np.broadcast_to(
            np.arange(P, dtype=np.float32).astype(ml_dtypes.bfloat16),
            (P, 16, P)).reshape(P, 16 * P))
    return _IOTA


def _pad_rows(a, n):
    out = np.zeros((n, a.shape[1]), dtype=a.dtype)
    out[:a.shape[0]] = a
    return out


def kernel(x1, x2, edge_index1, edge_index2, W1, b1, W2, b2):
    import ml_dtypes
    from concourse.bass_utils import run_bass_kernel_spmd

    bf16 = ml_dtypes.bfloat16
    x = [np.asarray(x1, np.float32).astype(bf16),
         np.asarray(x2, np.float32).astype(bf16)]
    W1 = np.asarray(W1, np.float32).astype(bf16)
    W2 = np.asarray(W2, np.float32).astype(bf16)
    b1 = np.asarray(b1, np.float32)
    b2 = np.asarray(b2, np.float32)
    cores, wcb, cnt16 = _host_prep(np.asarray(edge_index1), np.asarray(edge_index2))
    core_ids = list(range(N_CORES))

    # ---- launch A: h1' = dis (.) (x @ W1) per shard, bf16
    a_maps = []
    for c in cores:
        g, s = c["graph"], c["shard"]
        dis_sh = c["dis"][SHARD * s:SHARD * (s + 1), None]
        xs = _pad_rows(
            (x[g][SHARD * s:SHARD * (s + 1)].astype(np.float32) * dis_sh
             ).astype(x[g].dtype), SHARD_PAD)
        a_maps.append(dict(xT=np.ascontiguousarray(xs.T), w=W1, disb=c["disb"]))

    import time

    def _run(nc, maps, name):
        t0 = time.time()
        res = run_bass_kernel_spmd(nc, maps, core_ids)
        RUN_INFO.append(dict(name=name, wall_s=time.time() - t0,
                             exec_time_ns=res.exec_time_ns,
                             profile=res.profile_json))
        return res.results

    nc_a = _build_neff_a()
    a_out = _run(nc_a, a_maps, "A")

    def full_table(outs, key):
        tabs = []
        for g in range(2):
            shards = [outs[g * N_SHARDS + s][key][:SHARD] for s in range(N_SHARDS)]
            tabs.append(np.ascontiguousarray(np.concatenate(shards, axis=0)))
        return tabs

    h1 = full_table(a_out, "hout")

    # ---- launch B (x2): aggregation layers
    nc_b = _build_neff_b(wcb, cnt16)

    def run_b(htabs, bias):
        maps = []
        for c in cores:
            g, s = c["graph"], c["shard"]
            dis_sh = _pad_rows(
                c["dis"][SHARD * s:SHARD * (s + 1), None].astype(np.float32),
                SHARD_PAD)
            own = _pad_rows(htabs[g][SHARD * s:SHARD * (s + 1)], SHARD_PAD)
            h2v = (own.astype(np.float32) * dis_sh +
                   bias[None, :]).astype(own.dtype)
            maps.append(dict(
                htab=htabs[g],
                hot2=h2v,
                idx16=c["idx16"], dstloc=c["dstloc"], disb=c["disb"],
                iotaf=_iota_tile(), wnext=W2,
            ))
        run_b.n = getattr(run_b, "n", 0) + 1
        return _run(nc_b, maps, f"B{run_b.n}")

    b1_out = run_b(h1, b1)
    h2 = full_table(b1_out, "hnext")
    b2_out = run_b(h2, b2)
    emd = full_table(b2_out, "normout")
    return emd[0], emd[1]


# revision 14
# speedup vs baseline: 3.5048x; 1.0050x over previous
"""Trainium2 Bass kernel for a 2-layer GCN on two graphs (shared weights).

Problem: nn_BRIGHT_gcn (gnn_message_passing).
  reference:
    gcn_conv(x, ei, W, b): deg = 1 + indeg(col); dis = rsqrt(deg)
      h = x @ W; out[c] = sum_{(r,c) in E} dis[r]*dis[c]*h[r] + dis[c]^2*h[c] + b
    two layers, then L1-normalize rows.  Two graphs through the same weights.

Strategy (8 NeuronCores, SPMD):
  - graph g in {0,1} on cores 4g..4g+3; each core owns a contiguous shard of
    25000 destination nodes.
  - Factor the symmetric norm: h' = dis (.) (x @ W) stored as a bf16 table.
    The edge aggregation is a plain segment-sum of h'[src] rows, post-scaled
    by dis[dst]:  out = dis (.) (segsum(h'[src] -> dst) + h'[own]) + b
  - NEFF A: h1' = dis (.) (xT.T @ W1) in bf16 for the core's shard.
  - host: allgather h1' shards -> full bf16 table H' per graph (free).
  - NEFF B (compiled once, run twice): for each 128-dst block, gather the
    incoming edges' bf16 h' rows from HBM with dma_gather (int16 idxs, 4
    windows of 32768 rows, 4 SWDGE queues round-robin so Q7 descriptor
    generation overlaps ~2x) and scatter-add them with one-hot x PE matmuls
    (bf16, 1 cyc/row) into f32 PSUM.  Epilogue computes both
    h_next' = dis (.) ((out+b) @ W2) (bf16, the next layer's table) and
    l1norm(out+b) (f32); the host uses h_next' after layer 1 and l1norm
    after layer 2.
  - chunk counts are per-(block, window) maxima over the 8 cores (not a
    global max), cutting gather padding from +16% to +8%.

kernel() takes FULL inputs and returns the FULL output tuple.
"""

import math

import numpy as np

P = 128
FEAT = 256
N_NODES = 100000
N_CORES = 8
N_SHARDS = 4  # per graph
SHARD = N_NODES // N_SHARDS  # 25000
NBLK = math.ceil(SHARD / P)  # 196
SHARD_PAD = NBLK * P  # 25088
WIN = 32768  # int16 index window
N_WIN = math.ceil(N_NODES / WIN)  # 4
WIN_SIZES = [min(WIN, N_NODES - w * WIN) for w in range(N_WIN)]  # [32768]*3+[1696]
N_QUEUES = 4  # SWDGE descriptor queues (round-robin over gather calls)


# ---------------------------------------------------------------------------
# host-side graph preprocessing
# ---------------------------------------------------------------------------

def _prep_graph(edge_index):
    """Degree vector (with self-loops) for one graph."""
    col = np.asarray(edge_index[1], dtype=np.int64)
    deg = np.bincount(col, minlength=N_NODES).astype(np.float32) + 1.0
    dis = (1.0 / np.sqrt(deg)).astype(np.float32)
    return dis


def _prep_shard_edges(edge_index, shard_id):
    """Bucket one shard's incoming edges by (dst block, src window).

    Returns dict with per-(block, window) counts plus sorted per-edge arrays:
      blk   [e] destination block within shard (0..NBLK-1)
      dloc  [e] destination lane within block (0..127)
      widx  [e] source row within its window (0..32767)
      win   [e] source window (0..3)
      cnt   [NBLK, N_WIN] group sizes
    sorted by (blk, win), stable.
    """
    row = np.asarray(edge_index[0], dtype=np.int64)
    col = np.asarray(edge_index[1], dtype=np.int64)
    lo, hi = SHARD * shard_id, SHARD * (shard_id + 1)
    m = (col >= lo) & (col < hi)
    src = row[m]
    dst = col[m] - lo
    blk = dst >> 7
    dloc = dst & 127
    win = src >> 15
    widx = src & (WIN - 1)
    # widx as the innermost key: ascending row addresses within each gather
    # group make the random reads quasi-sequential in HBM (row-buffer hits)
    order = np.lexsort((widx, win, blk))
    blk, dloc, win, widx = blk[order], dloc[order], win[order], widx[order]
    cnt = np.bincount(blk * N_WIN + win, minlength=NBLK * N_WIN).reshape(NBLK, N_WIN)
    return dict(blk=blk, dloc=dloc, win=win, widx=widx, cnt=cnt)


def _build_core_tables(sh, wcb):
    """Build the per-core device-side index/onehot tables.

    wcb[b, w]: chunks (of 128 edges) allotted to the (block b, window w)
    group — per-(b, w) maxima over cores, shared by the single NEFF.
    Flat free-dim layout per block b:
      idx16 : for w in 0..3: wcb[b,w]*128 int16 window-row indices, wrapped
              [16, L/16] (pos j -> partition j%16, slot j//16), replicated to
              128 partitions.
      dstloc: for w in 0..3: wcb[b,w] columns of 128 bf16 dst lanes (pad=-1).
    Padding edges gather window row 0 and have dstloc -1 (one-hot zero).
    """
    import ml_dtypes

    blk, win, widx, dloc, cnt = sh["blk"], sh["win"], sh["widx"], sh["dloc"], sh["cnt"]
    L = wcb * P  # [NBLK, N_WIN] padded group sizes
    # free-dim offset of group (b, w) in the flat per-edge layout
    flat_sizes = L.reshape(-1)
    gstart = np.zeros(NBLK * N_WIN, dtype=np.int64)
    gstart[1:] = np.cumsum(flat_sizes)[:-1]
    tot = int(flat_sizes.sum())

    flat_cnt = cnt.reshape(-1)
    estart = np.zeros(NBLK * N_WIN, dtype=np.int64)
    estart[1:] = np.cumsum(flat_cnt)[:-1]
    gid = blk * N_WIN + win
    pos_in_group = np.arange(len(blk)) - estart[gid]
    flat_pos = gstart[gid] + pos_in_group

    idx_flat = np.zeros(tot, dtype=np.int16)
    dloc_flat = np.full(tot, -1.0, dtype=np.float32)
    idx_flat[flat_pos] = widx.astype(np.int16)
    dloc_flat[flat_pos] = dloc.astype(np.float32)

    # idx16: per (b, w) wrap [L] -> [16, L/16]; concat along free dim;
    # replicate to 128 partitions.
    idx_parts = []
    dl_parts = []
    for b in range(NBLK):
        for w in range(N_WIN):
            g0 = gstart[b * N_WIN + w]
            seg = idx_flat[g0:g0 + L[b, w]]
            idx_parts.append(seg.reshape(L[b, w] // 16, 16).T)  # [16, L/16]
            dl_parts.append(dloc_flat[g0:g0 + L[b, w]].reshape(wcb[b, w], P).T)
    idx16 = np.concatenate(idx_parts, axis=1)  # [16, tot/16]
    idx16 = np.tile(idx16, (8, 1))  # [128, tot/16]
    dstloc = np.concatenate(dl_parts, axis=1).astype(ml_dtypes.bfloat16)
    return (np.ascontiguousarray(idx16), np.ascontiguousarray(dstloc))


def _dis_cols(dis, shard_id):
    """dis for the shard as [128, NBLK] (partition = lane in block)."""
    d = np.zeros(SHARD_PAD, dtype=np.float32)
    d[:SHARD] = dis[SHARD * shard_id:SHARD * (shard_id + 1)]
    return np.ascontiguousarray(d.reshape(NBLK, P).T)


def _host_prep(edge_index1, edge_index2):
    """All static per-core structures. Returns (cores, wcb)."""
    dis = [_prep_graph(edge_index1), _prep_graph(edge_index2)]
    shards = []
    for g, ei in enumerate((edge_index1, edge_index2)):
        for s in range(N_SHARDS):
            shards.append((g, s, _prep_shard_edges(ei, s)))
    # per-(block, window) chunk counts: max over the 8 cores
    cnt_max = np.stack([sh["cnt"] for _, _, sh in shards]).max(axis=0)
    wcb = np.maximum(1, np.ceil(cnt_max / P).astype(np.int64))  # [NBLK, N_WIN]
    cnt16 = np.maximum(16, np.ceil(cnt_max / 16).astype(np.int64) * 16)
    cores = []
    for g, s, sh in shards:
        idx16, dstloc = _build_core_tables(sh, wcb)
        cores.append(dict(
            graph=g, shard=s,
            idx16=idx16, dstloc=dstloc,
            disb=_dis_cols(dis[g], s),
            dis=dis[g],
        ))
    return cores, wcb, cnt16


# ---------------------------------------------------------------------------
# device kernels (bass/tile)
# ---------------------------------------------------------------------------

def _build_neff_a():
    import concourse.bacc as bacc
    import concourse.mybir as mybir
    import concourse.tile as tile

    f32 = mybir.dt.float32
    bf16 = mybir.dt.bfloat16
    nc = bacc.Bacc("TRN2", target_bir_lowering=False, debug=False)
    xT = nc.dram_tensor("xT", [FEAT, SHARD_PAD], bf16, kind="ExternalInput")
    w_in = nc.dram_tensor("w", [FEAT, FEAT], bf16, kind="ExternalInput")
    disb = nc.dram_tensor("disb", [P, NBLK], f32, kind="ExternalInput")
    hout = nc.dram_tensor("hout", [SHARD_PAD, FEAT], bf16, kind="ExternalOutput")

    with tile.TileContext(nc) as tc:
        with (
            tc.tile_pool(name="const", bufs=1) as cpool,
            tc.tile_pool(name="work", bufs=3) as wpool,
            tc.tile_pool(name="psum", bufs=2, space="PSUM") as ppool,
        ):
            w_sb = cpool.tile([P, 2, FEAT], bf16, tag="w")
            nc.sync.dma_start(out=w_sb[:, 0, :], in_=w_in[0:P, :])
            nc.sync.dma_start(out=w_sb[:, 1, :], in_=w_in[P:FEAT, :])
            dis_sb = cpool.tile([P, NBLK], f32, tag="dis")
            nc.sync.dma_start(out=dis_sb[:], in_=disb[:, :])
            for t in range(NBLK):
                xt = wpool.tile([P, 2, P], bf16, tag="xT")
                nc.sync.dma_start(out=xt[:, 0, :], in_=xT[0:P, t * P:(t + 1) * P])
                nc.sync.dma_start(out=xt[:, 1, :], in_=xT[P:FEAT, t * P:(t + 1) * P])
                ps = ppool.tile([P, FEAT], f32, tag="ps")
                nc.tensor.matmul(ps[:], lhsT=xt[:, 0, :], rhs=w_sb[:, 0, :],
                                 start=True, stop=False)
                nc.tensor.matmul(ps[:], lhsT=xt[:, 1, :], rhs=w_sb[:, 1, :],
                                 start=False, stop=True)
                # dis is folded into xT on the host: h' = (dis (.) x) @ W
                hs = wpool.tile([P, FEAT], bf16, tag="hs")
                nc.vector.tensor_scalar(out=hs[:], in0=ps[:], scalar1=1.0,
                                        scalar2=None, op0=mybir.AluOpType.mult)
                nc.sync.dma_start(out=hout[t * P:(t + 1) * P, :], in_=hs[:])
    nc.compile()
    return nc


def _build_neff_b(wcb, cnt16):
    import concourse.bacc as bacc
    import concourse.mybir as mybir
    import concourse.tile as tile
    from concourse.masks import make_identity

    f32 = mybir.dt.float32
    bf16 = mybir.dt.bfloat16
    i16 = mybir.dt.int16
    SIDXb = [int(wcb[b].sum()) * 8 for b in range(NBLK)]  # int16 cols per block
    NCHb = [int(wcb[b].sum()) for b in range(NBLK)]  # chunks per block
    TOT_SIDX = sum(SIDXb)
    TOT_NCH = sum(NCHb)
    SIDX_MAX = max(SIDXb)
    NCH_MAX = max(NCHb)
    nc = bacc.Bacc("TRN2", target_bir_lowering=False, debug=False,
                   num_swdge_queues=N_QUEUES)
    htab = nc.dram_tensor("htab", [N_NODES, FEAT], bf16, kind="ExternalInput")
    # hot2 = dis (.) h'own + b  (self-loop term + bias, host-precomputed)
    hot2 = nc.dram_tensor("hot2", [SHARD_PAD, FEAT], bf16, kind="ExternalInput")
    idx16 = nc.dram_tensor("idx16", [P, TOT_SIDX], i16, kind="ExternalInput")
    dstloc = nc.dram_tensor("dstloc", [P, TOT_NCH], bf16, kind="ExternalInput")
    disb = nc.dram_tensor("disb", [P, NBLK], f32, kind="ExternalInput")
    OHG = 16  # one-hot chunks generated per DVE op
    iotaf = nc.dram_tensor("iotaf", [P, OHG * P], bf16, kind="ExternalInput")
    wnext = nc.dram_tensor("wnext", [FEAT, FEAT], bf16, kind="ExternalInput")
    hnext = nc.dram_tensor("hnext", [SHARD_PAD, FEAT], bf16,
                           kind="ExternalOutput")
    normout = nc.dram_tensor("normout", [SHARD_PAD, FEAT], f32,
                             kind="ExternalOutput")

    qc = [0]  # SWDGE queue rotation counter

    def next_q():
        q = qc[0] % N_QUEUES
        qc[0] += 1
        return q

    with tile.TileContext(nc) as tc:
        with (
            tc.tile_pool(name="const", bufs=1) as cpool,
            tc.tile_pool(name="gland", bufs=2) as gpool,
            tc.tile_pool(name="work", bufs=3) as wpool,
            tc.tile_pool(name="oh", bufs=6) as ohpool,
            tc.tile_pool(name="psum", bufs=2, space="PSUM") as ppool,
            tc.tile_pool(name="psumt", bufs=2, space="PSUM") as ptpool,
        ):
            w_sb = cpool.tile([P, 2, FEAT], bf16, tag="w")
            nc.sync.dma_start(out=w_sb[:, 0, :], in_=wnext[0:P, :])
            nc.sync.dma_start(out=w_sb[:, 1, :], in_=wnext[P:FEAT, :])
            dis_sb = cpool.tile([P, NBLK], f32, tag="dis")
            nc.sync.dma_start(out=dis_sb[:], in_=disb[:, :])
            iota_sb = cpool.tile([P, OHG, P], bf16, tag="iota")
            nc.sync.dma_start(out=iota_sb[:], in_=iotaf[:, :].rearrange(
                "p (g q) -> p g q", g=OHG))
            dl_sb = cpool.tile([P, TOT_NCH], bf16, tag="dl")
            nc.sync.dma_start(out=dl_sb[:], in_=dstloc[:, :])
            ident = cpool.tile([P, P], bf16, tag="ident")
            make_identity(nc, ident[:])

            ioff_b = 0  # running idx16 free-dim offset
            coff_b = 0  # running dstloc chunk-col offset
            for b in range(NBLK):
                SIDX = SIDXb[b]
                NCH = NCHb[b]
                idxt = wpool.tile([P, SIDX_MAX], i16, tag="idx")
                nc.sync.dma_start(out=idxt[:, 0:SIDX],
                                  in_=idx16[:, ioff_b:ioff_b + SIDX])
                gt = gpool.tile([P, NCH_MAX, FEAT], bf16, tag="g")
                ioff_w = 0
                goff = 0
                for w in range(N_WIN):
                    wc = int(wcb[b, w])
                    n16 = int(cnt16[b, w])  # 16-padded max edge count
                    src = htab[w * WIN:w * WIN + WIN_SIZES[w], :]
                    for p0 in range(0, wc, 8):
                        pc = min(8, wc - p0)
                        nq_rows = min(pc * P, n16 - p0 * P)
                        nc.gpsimd.dma_gather(
                            gt[:, goff + p0:goff + p0 + pc, :],
                            src,
                            idxt[:, ioff_w + p0 * 8:
                                 ioff_w + p0 * 8 + nq_rows // 16],
                            nq_rows,
                            nq_rows,
                            FEAT,
                            single_packet=False,
                            queue_num=next_q(),
                        )
                    ioff_w += wc * 8
                    goff += wc
                ps = ppool.tile([P, FEAT], f32, tag="agg")
                # batched one-hot generation: OHG chunks per DVE op
                ohs = []
                for c0 in range(0, NCH, OHG):
                    g = min(OHG, NCH - c0)
                    oh = ohpool.tile([P, OHG, P], bf16, tag="oh")
                    nc.vector.tensor_tensor(
                        out=oh[:, :g, :], in0=iota_sb[:, :g, :],
                        in1=dl_sb[:, coff_b + c0:coff_b + c0 + g]
                        .to_broadcast([P, g, P]),
                        op=mybir.AluOpType.is_equal)
                    ohs.append(oh)
                for j in range(NCH):
                    oh = ohs[j // OHG][:, j % OHG, :]
                    nc.tensor.matmul(ps[:], lhsT=oh, rhs=gt[:, j, :],
                                     start=(j == 0), stop=(j == NCH - 1))
                hot = wpool.tile([P, FEAT], bf16, tag="hot")
                nc.sync.dma_start(out=hot[:], in_=hot2[b * P:(b + 1) * P, :])
                # t2 = dis (.) ps on the (otherwise idle) scalar engine
                t2 = wpool.tile([P, FEAT], f32, tag="t2")
                nc.scalar.activation(out=t2[:], in_=ps[:],
                                     func=mybir.ActivationFunctionType.Copy,
                                     scale=dis_sb[:, b:b + 1])
                # ob = layer output in bf16 (= t2 + hot2)
                ob = wpool.tile([P, FEAT], bf16, tag="ob")
                nc.vector.tensor_tensor(out=ob[:], in0=t2[:], in1=hot[:],
                                        op=mybir.AluOpType.add)
                # --- h_next branch: dis (.) (ob @ W2), bf16
                tp = ptpool.tile([P, 2, P], bf16, tag="tp")
                nc.tensor.transpose(tp[:, 0, :], ob[:, 0:P], ident[:])
                nc.tensor.transpose(tp[:, 1, :], ob[:, P:FEAT], ident[:])
                tts = wpool.tile([P, 2, P], bf16, tag="tts")
                nc.vector.tensor_copy(tts[:, 0, :], tp[:, 0, :])
                nc.scalar.copy(tts[:, 1, :], tp[:, 1, :])
                ps2 = ppool.tile([P, FEAT], f32, tag="mm2")
                nc.tensor.matmul(ps2[:], lhsT=tts[:, 0, :], rhs=w_sb[:, 0, :],
                                 start=True, stop=False)
                nc.tensor.matmul(ps2[:], lhsT=tts[:, 1, :], rhs=w_sb[:, 1, :],
                                 start=False, stop=True)
                hn = wpool.tile([P, FEAT], bf16, tag="hn")
                nc.scalar.activation(out=hn[:], in_=ps2[:],
                                     func=mybir.ActivationFunctionType.Copy,
                                     scale=dis_sb[:, b:b + 1])
                nc.sync.dma_start(out=hnext[b * P:(b + 1) * P, :], in_=hn[:])
                # --- l1 normalize branch
                s1 = wpool.tile([P, 1], f32, tag="s1")
                nc.vector.tensor_reduce(out=s1[:], in_=ob[:],
                                        axis=mybir.AxisListType.X,
                                        op=mybir.AluOpType.add,
                                        apply_absolute_value=True)
                s2 = wpool.tile([P, 1], f32, tag="s2")
                nc.vector.tensor_scalar(out=s2[:], in0=s1[:], scalar1=1e-12,
                                        scalar2=None, op0=mybir.AluOpType.max)
                rs = wpool.tile([P, 1], f32, tag="rs")
                nc.vector.reciprocal(rs[:], s2[:])
                no = wpool.tile([P, FEAT], f32, tag="no")
                nc.scalar.activation(out=no[:], in_=ob[:],
                                     func=mybir.ActivationFunctionType.Copy,
                                     scale=rs[:, 0:1])
                nc.sync.dma_start(out=normout[b * P:(b + 1) * P, :], in_=no[:])
                ioff_b += SIDX
                coff_b += NCH
    nc.compile()
    return nc


# ---------------------------------------------------------------------------
# orchestration
# ---------------------------------------------------------------------------

RUN_INFO = []  # per-launch {name, wall_s, exec_time_ns} (exec only when traced)

_IOTA = None


def _iota_tile():
    global _IOTA
    if _IOTA is None:
        import ml_dtypes
        _IOTA = np.ascontiguousarray(np.broadcast_to(
            np.arange(P, dtype=np.float32).astype(ml_dtypes.bfloat16),
            (P, 16, P)).reshape(P, 16 * P))
    return _IOTA


def _pad_rows(a, n):
    out = np.zeros((n, a.shape[1]), dtype=a.dtype)
    out[:a.shape[0]] = a
    return out


def kernel(x1, x2, edge_index1, edge_index2, W1, b1, W2, b2):
    import ml_dtypes
    from concourse.bass_utils import run_bass_kernel_spmd

    bf16 = ml_dtypes.bfloat16
    x = [np.asarray(x1, np.float32).astype(bf16),
         np.asarray(x2, np.float32).astype(bf16)]
    W1 = np.asarray(W1, np.float32).astype(bf16)
    W2 = np.asarray(W2, np.float32).astype(bf16)
    b1 = np.asarray(b1, np.float32)
    b2 = np.asarray(b2, np.float32)
    cores, wcb, cnt16 = _host_prep(np.asarray(edge_index1), np.asarray(edge_index2))
    core_ids = list(range(N_CORES))

    # ---- launch A: h1' = dis (.) (x @ W1) per shard, bf16
    a_maps = []
    for c in cores:
        g, s = c["graph"], c["shard"]
        dis_sh = c["dis"][SHARD * s:SHARD * (s + 1), None]
        xs = _pad_rows(
            (x[g][SHARD * s:SHARD * (s + 1)].astype(np.float32) * dis_sh
             ).astype(x[g].dtype), SHARD_PAD)
        a_maps.append(dict(xT=np.ascontiguousarray(xs.T), w=W1, disb=c["disb"]))

    import time

    def _run(nc, maps, name):
        t0 = time.time()
        res = run_bass_kernel_spmd(nc, maps, core_ids)
        RUN_INFO.append(dict(name=name, wall_s=time.time() - t0,
                             exec_time_ns=res.exec_time_ns,
                             profile=res.profile_json))
        return res.results

    nc_a = _build_neff_a()
    a_out = _run(nc_a, a_maps, "A")

    def full_table(outs, key):
        tabs = []
        for g in range(2):
            shards = [outs[g * N_SHARDS + s][key][:SHARD] for s in range(N_SHARDS)]
            tabs.append(np.ascontiguousarray(np.concatenate(shards, axis=0)))
        return tabs

    h1 = full_table(a_out, "hout")

    # ---- launch B (x2): aggregation layers
    nc_b = _build_neff_b(wcb, cnt16)

    def run_b(htabs, bias):
        maps = []
        for c in cores:
            g, s = c["graph"], c["shard"]
            dis_sh = _pad_rows(
                c["dis"][SHARD * s:SHARD * (s + 1), None].astype(np.float32),
                SHARD_PAD)
            own = _pad_rows(htabs[g][SHARD * s:SHARD * (s + 1)], SHARD_PAD)
            h2v = (own.astype(np.float32) * dis_sh +
                   bias[None, :]).astype(own.dtype)
            maps.append(dict(
                htab=htabs[g],
                hot2=h2v,
                idx16=c["idx16"], dstloc=c["dstloc"], disb=c["disb"],
                iotaf=_iota_tile(), wnext=W2,
            ))
        run_b.n = getattr(run_b, "n", 0) + 1
        return _run(nc_b, maps, f"B{run_b.n}")

    b1_out = run_b(h1, b1)
    h2 = full_table(b1_out, "hnext")
    b2_out = run_b(h2, b2)
    emd = full_table(b2_out, "normout")
    return emd[0], emd[1]
